# revision 51
# baseline (speedup 1.0000x reference)
"""Trainium2 Bass kernel for DenseCGPrior (PaiNN-style CG message passing).

Self-contained: hardcodes B=4, N=128, F=128, N_RBF=20, CUTOFF=5.0, L=3.
Sharding: data-parallel over batch; core c computes batch c % 4 (cores 4-7
duplicate so all 8 cores run the same SPMD program).

Key restructuring vs the reference: the [N,N,3F] per-edge message tensor is
never materialized. With 21 radial channels (20 RBF + 1 bias carrying rbf_b),
   inv[i,j,f'] = phi[j,f'] * sum_k g_k[i,j] * W[k,f']
so each edge reduction becomes 21 PSUM-accumulated matmuls with the symmetric
geometry matrices G_k (or G_k/d for the unit-vector term, which is decomposed
via unit = (x_j - x_i)/d into two matmul families plus rank-1 corrections).
"""

import os
import sys

import numpy as np

for _p in ("/opt/trn_rl_repo", "/root/.axon_site/_ro/trn_rl_repo"):
    if os.path.isdir(_p) and _p not in sys.path:
        sys.path.insert(0, _p)

import concourse.bass as bass
import concourse.mybir as mybir
import concourse.tile as tile
from concourse.masks import make_identity

F32 = mybir.dt.float32
F32R = mybir.dt.float32r
AF = mybir.ActivationFunctionType
OP = mybir.AluOpType

B, N, F, NRBF, L = 4, 128, 128, 20, 3
KC = NRBF + 1            # rbf channels + bias channel
KCH = 7                  # k-chunk size (21 = 3 chunks of 7)
F3 = 3 * F
EPS = 0.001
PI = float(np.pi)
CUTOFF = 5.0
N_CORES = 8

_IN_SPECS = [
    ("H", [N, F]),
    ("adj", [N, N]),
    ("xyz", [N, 3]),
    ("wbc", [L, 3, N, 3, KCH, F]),  # folded rbf weights, k-chunked,
                                    # replicated per partition
    ("freqn", [N, NRBF]),        # freq_k/(2pi) replicated per partition
    ("mw1", [F, L, F]),          # msg_W1 transposed to [f_in, l, f_out]
    ("mb1", [F, L]),
    ("mw2", [F, L, F3]),
    ("mb2", [F, L, 3]),          # [f, l, part]
    ("uwu", [F, L, F]),
    ("uwv", [F, L, F]),
    ("uws1", [F, L, 2, F]),      # [row_in_chunk, l, chunk, f_out]
    ("ubs1", [F, L]),
    ("uws2", [F, L, F3]),
    ("ubs2", [F, L, 3]),
    ("hmw1", [F, F]), ("hmb1", [F, 1]),
    ("hmw2", [F, F]), ("hmb2", [F, 1]),
    ("hsw1", [F, F]), ("hsb1", [F, 1]),
    ("hsw2", [F, F]), ("hsb2", [F, 1]),
]


def _rep(ap, times):
    """Read-broadcast a [P, M] AP as [P, times, M] via a step-0 free dim."""
    return bass.AP(tensor=ap.tensor, offset=ap.offset,
                   ap=[ap.ap[0], [0, times], *ap.ap[1:]])


def build_program(nc):
    dins = {name: nc.dram_tensor(name, shape, F32, kind="ExternalInput")
            for name, shape in _IN_SPECS}
    out_mu = nc.dram_tensor("out_mu", [N, F], F32, kind="ExternalOutput")
    out_sig = nc.dram_tensor("out_sig", [N, F], F32, kind="ExternalOutput")

    from contextlib import ExitStack
    with tile.TileContext(nc) as tc, ExitStack() as ctx:
        consts = ctx.enter_context(tc.tile_pool(name="consts", bufs=1))
        geom = ctx.enter_context(tc.tile_pool(name="geom", bufs=1))
        state = ctx.enter_context(tc.tile_pool(name="state", bufs=1))
        work = ctx.enter_context(tc.tile_pool(name="work", bufs=2))
        fams = ctx.enter_context(tc.tile_pool(name="fams", bufs=2))
        wbcp = ctx.enter_context(tc.tile_pool(name="wbcp", bufs=6))
        pp = ctx.enter_context(tc.tile_pool(name="pp", bufs=2, space="PSUM"))

        # ---- constants / weights to SBUF ----
        ident = consts.tile([N, N], F32)
        make_identity(nc, ident[:])
        # PE warmup on ident: absorbs the Pool-sem wait so later transposes
        # carry at most one sync wait (walrus LW struct limit).
        ps_wu = pp.tile([N, N], F32, tag="tr")
        nc.tensor.transpose(ps_wu[:], ident[:], ident[:])

        sb = {}
        _early = ("xyz", "adj", "freqn", "mw1", "mb1", "mw2", "mb2")

        def _load(names):
            for name, shape in _IN_SPECS:
                if name in ("H", "wbc") or name in sb:
                    continue
                if names is not None and name not in names:
                    continue
                t = consts.tile(shape, F32, tag=f"w_{name}")
                nc.sync.dma_start(t[:], dins[name].ap())
                sb[name] = t

        # wbc chunk 0 of layer 0 gates the first message matmuls: issue its
        # DMA before anything else.
        wbc0 = []
        for ch in range(KC // KCH):
            wb = wbcp.tile([N, 3, KCH, F], F32, tag="wbc")
            nc.sync.dma_start(wb[:], dins["wbc"].ap()[0, ch])
            wbc0.append(wb)
        _load(_early)

        def ccol(val, tag):
            t = consts.tile([N, 1], F32, tag=tag)
            nc.vector.memset(t[:], val)
            return t

        c_eps = ccol(EPS, "c_eps")
        c_halfpi = ccol(PI / 2, "c_halfpi")

        def silu(out_t, in_ps, biascol, ztag):
            z = work.tile([F, N], F32, tag=ztag)
            nc.vector.tensor_scalar_add(z[:], in_ps, biascol)
            nc.scalar.activation(out_t, z[:], AF.Sigmoid)
            nc.vector.tensor_mul(out_t, z[:], out_t)

        # ---- geometry ----
        xyzt = sb["xyz"]
        adjt = sb["adj"]

        # xyzT [3, N] and -2*xyzT
        xyz_s = geom.tile([N, 3], F32)
        nc.vector.tensor_copy(xyz_s[:], xyzt[:])
        ps_x = pp.tile([3, N], F32, tag="tr")
        nc.tensor.transpose(ps_x[:], xyz_s[:], ident[:])
        xyzT = geom.tile([3, N], F32)
        nc.vector.tensor_copy(xyzT[:], ps_x[:])
        xyzTm2 = geom.tile([3, N], F32)
        nc.scalar.mul(xyzTm2[:], xyzT[:], -2.0)
        sqT = geom.tile([3, N], F32)
        nc.vector.tensor_mul(sqT[:], xyzT[:], xyzT[:])
        ones31 = geom.tile([3, 1], F32)
        nc.vector.memset(ones31[:], 1.0)
        ones1N = geom.tile([1, N], F32)
        nc.vector.memset(ones1N[:], 1.0)
        ps_nn = pp.tile([1, N], F32, tag="tr")
        nc.tensor.matmul(ps_nn[:], ones31[:], sqT[:], start=True, stop=True)
        nn_row = geom.tile([1, N], F32)
        nc.vector.tensor_copy(nn_row[:], ps_nn[:])

        # d2 = |xi|^2 + |xj|^2 - 2<xi,xj>  (PSUM accumulation)
        ps_d2 = pp.tile([N, N], F32, tag="tr")
        nc.tensor.matmul(ps_d2[:], xyzTm2[:], xyzT[:], start=True, stop=False)
        nc.tensor.matmul(ps_d2[:], ones1N[:], nn_row[:], start=False, stop=False)
        nc.tensor.matmul(ps_d2[:], nn_row[:], ones1N[:], start=False, stop=True)
        dmat = geom.tile([N, N], F32)
        nc.scalar.activation(dmat[:], ps_d2[:], AF.Sqrt, bias=c_eps[:])
        invd = geom.tile([N, N], F32)
        nc.vector.reciprocal(invd[:], dmat[:])

        # deg / dis / ew
        deg = geom.tile([N, 1], F32)
        nc.vector.reduce_sum(deg[:], adjt[:], axis=mybir.AxisListType.X)
        dis = geom.tile([N, 1], F32)
        nc.vector.reciprocal(dis[:], deg[:])
        nc.scalar.activation(dis[:], dis[:], AF.Sqrt, bias=c_eps[:])
        dis_s = geom.tile([N, 1], F32)
        nc.vector.tensor_copy(dis_s[:], dis[:])
        ps_dr = pp.tile([1, N], F32, tag="tr")
        nc.tensor.transpose(ps_dr[:], dis_s[:], ident[:])
        dis_row = geom.tile([1, N], F32)
        nc.vector.tensor_copy(dis_row[:], ps_dr[:])
        ps_ew = pp.tile([N, N], F32, tag="tr")
        nc.tensor.matmul(ps_ew[:], dis_row[:], dis_row[:], start=True, stop=True)
        mask = geom.tile([N, N], F32)
        nc.vector.tensor_scalar(mask[:], adjt[:], 0.0, None, op0=OP.is_gt)
        ew = geom.tile([N, N], F32)
        nc.vector.tensor_mul(ew[:], mask[:], ps_ew[:])

        # envelope: env = 0.5 + 0.5*sin(pi/2 - pi*min(d,CUTOFF)/CUTOFF)
        dc = geom.tile([N, N], F32)
        nc.vector.tensor_scalar_min(dc[:], dmat[:], CUTOFF)
        env = geom.tile([N, N], F32)
        nc.scalar.activation(env[:], dc[:], AF.Sin,
                             bias=c_halfpi[:], scale=-PI / CUTOFF)
        nc.vector.tensor_scalar(env[:], env[:], 0.5, 0.5,
                                op0=OP.mult, op1=OP.add)
        eew = geom.tile([N, N], F32)
        nc.vector.tensor_mul(eew[:], env[:], ew[:])
        sh1 = geom.tile([N, N], F32)    # env*ew/d
        nc.vector.tensor_mul(sh1[:], eew[:], invd[:])

        # G[j, k, i]: k<20 -> sin(d*freq_k)*env*ew/d ; k=20 -> env*ew
        # sin range reduction: t = d*freq_k/(2pi); frac = t - rne(t) in
        # [-1/2, 1/2]; sin(d*freq_k) = sin(2pi*frac). rne via the exact
        # float trick (t + 1.5*2^23) - 1.5*2^23, identical on DVE and numpy.
        G = geom.tile([N, KC, N], F32R)
        GD = geom.tile([N, KC, N], F32R)
        RC = 12582912.0  # 1.5 * 2^23
        fq = sb["freqn"]  # [N, NRBF] per-partition copies of freq_k/(2pi)
        fq_ap = bass.AP(tensor=fq[:].tensor, offset=fq[:].offset,
                        ap=[fq[:].ap[0], fq[:].ap[1], [0, N]])
        # Built in k-chunks of 7 so the first message matmuls can start as
        # soon as chunk 0 is ready. f32 scratches borrowed from the fam pool
        # (distinct memrefs, so the fp32r producer-rounding check is happy).
        for ch in range(KC // KCH):
            k0 = ch * KCH
            nk = min(KCH, NRBF - k0)  # last chunk: 6 rbf rows + bias row
            scr1 = fams.tile([N, KCH, N], F32, tag="fam1")
            scr2 = fams.tile([N, KCH, N], F32, tag="fam2")
            fqa = bass.AP(tensor=fq[:].tensor, offset=fq[:].offset + k0,
                          ap=[fq[:].ap[0], [1, nk], [0, N]])
            nc.vector.tensor_tensor(scr1[:, 0:nk, :], _rep(dmat[:], nk),
                                    fqa, op=OP.mult)
            nc.vector.tensor_scalar(scr2[:, 0:nk, :], scr1[:, 0:nk, :],
                                    RC, RC, op0=OP.add, op1=OP.subtract)
            nc.vector.tensor_tensor(scr1[:, 0:nk, :], scr1[:, 0:nk, :],
                                    scr2[:, 0:nk, :], op=OP.subtract)
            nc.scalar.activation(scr1[:, 0:nk, :], scr1[:, 0:nk, :],
                                 AF.Sin, scale=2.0 * PI)
            nc.vector.tensor_tensor(G[:, k0:k0 + nk, :], scr1[:, 0:nk, :],
                                    _rep(sh1[:], nk), op=OP.mult)
            if ch == KC // KCH - 1:
                nc.vector.tensor_copy(G[:, NRBF, :], eew[:])
            nc.vector.tensor_tensor(GD[:, k0:k0 + KCH, :],
                                    G[:, k0:k0 + KCH, :].bitcast(F32),
                                    _rep(invd[:], KCH), op=OP.mult)

        # ---- initial state ----
        sT = state.tile([F, N], F32)       # s transposed [f, n]
        ps_s0 = pp.tile([N, N], F32, tag="tr")
        h0 = work.tile([N, F], F32, tag="h0")
        nc.sync.dma_start(h0[:], dins["H"].ap())
        h0s = work.tile([N, F], F32, tag="h0s")
        nc.vector.tensor_copy(h0s[:], h0[:])
        nc.tensor.transpose(ps_s0[:], h0s[:], ident[:])
        nc.vector.tensor_copy(sT[:], ps_s0[:])
        v = state.tile([N, 3, F], F32)     # v[n, c, f]

        def load_wbc(l):
            """DMA layer-l folded rbf weights (host-replicated), k-chunked."""
            wbs = []
            for ch in range(KC // KCH):
                wb = wbcp.tile([N, 3, KCH, F], F32, tag="wbc")
                nc.sync.dma_start(wb[:], dins["wbc"].ap()[l, ch])
                wbs.append(wb)
            return wbs

        _load(None)  # remaining weights, lower DMA priority

        for l in range(L):
            wbc = wbc0 if l == 0 else load_wbc(l)
            # ---- phi = silu(s@W1 + b1) @ W2 + b2, produced transposed ----
            ps1 = pp.tile([F, N], F32, tag="mm")
            nc.tensor.matmul(ps1[:], sb["mw1"][:, l, :], sT[:],
                             start=True, stop=True)
            act1 = work.tile([F, N], F32, tag="act1")
            silu(act1[:], ps1[:], sb["mb1"][:, l:l + 1], "z1")
            phi = [None, None, None]
            for p in (2, 1, 0):   # part 2 first: it gates the o2 group
                ps2 = pp.tile([F, N], F32, tag="mm")
                nc.tensor.matmul(ps2[:], sb["mw2"][:, l, p * F:(p + 1) * F],
                                 act1[:], start=True, stop=True)
                phiT_p = work.tile([F, N], F32, tag=f"phiT{p}")
                nc.vector.tensor_scalar_add(phiT_p[:], ps2[:],
                                            sb["mb2"][:, l, p:p + 1])
                ps_t = pp.tile([N, F], F32, tag="tr")
                nc.tensor.transpose(ps_t[:], phiT_p[:], ident[:])
                phi_p = work.tile([N, F], F32, tag=f"phi{p}")
                nc.vector.tensor_copy(phi_p[:], ps_t[:])
                phi[p] = phi_p

            pv = []
            if l > 0:
                for c in range(3):
                    pv_c = work.tile([N, F], F32, tag=f"pv{c}")
                    nc.vector.tensor_mul(pv_c[:], phi[0][:], v[:, c, :])
                    pv.append(pv_c)

            # ---- message matmuls: 21 channels in 3 chunks of 7 ----
            nslot1 = 1 if l == 0 else 4
            o1 = pp.tile([N, nslot1 * F], F32, tag="acc")
            o2 = pp.tile([N, 4 * F], F32, tag="acc")
            for ch in range(KC // KCH):
                k0 = ch * KCH
                wb = wbc[ch]
                fam2 = fams.tile([N, KCH, 4, F], F32R, tag="fam2")
                nc.vector.tensor_tensor(fam2[:, :, 0, :],
                                        _rep(phi[2][:], KCH),
                                        wb[:, 2, :, :], op=OP.mult)
                for c in range(3):
                    nc.scalar.mul(fam2[:, :, 1 + c, :], fam2[:, :, 0, :],
                                  xyzt[:, c:c + 1])
                fam1 = fams.tile([N, KCH, nslot1, F], F32R, tag="fam1")
                nc.gpsimd.tensor_tensor(fam1[:, :, 0, :],
                                        _rep(phi[1][:], KCH),
                                        wb[:, 1, :, :], op=OP.mult)
                for c in range(3) if l > 0 else ():
                    nc.vector.tensor_tensor(fam1[:, :, 1 + c, :],
                                            _rep(pv[c][:], KCH),
                                            wb[:, 0, :, :],
                                            op=OP.mult)
                for kk in range(KCH):
                    k = k0 + kk
                    nc.tensor.matmul(
                        o2[:], GD[:, k, :],
                        fam2[:, kk, :, :].rearrange("p s f -> p (s f)"),
                        start=(k == 0), stop=(k == KC - 1))
                for kk in range(KCH):
                    k = k0 + kk
                    nc.tensor.matmul(
                        o1[:], G[:, k, :],
                        fam1[:, kk, :, :].rearrange("p s f -> p (s f)"),
                        start=(k == 0), stop=(k == KC - 1))

            # ---- apply ds / dv ----
            ds = work.tile([N, F], F32, tag="ds")
            nc.vector.tensor_copy(ds[:], o1[:, 0:F])
            ps_dst = pp.tile([F, N], F32, tag="tr")
            nc.tensor.transpose(ps_dst[:], ds[:], ident[:])
            nc.vector.tensor_add(sT[:], sT[:], ps_dst[:])

            for c in range(3):
                qx = work.tile([N, F], F32, tag=f"qx{c}")
                nc.vector.tensor_scalar_mul(qx[:], o2[:, 0:F],
                                            xyzt[:, c:c + 1])
                lo = (1 + c) * F
                if l == 0:
                    nc.vector.tensor_sub(v[:, c, :], o2[:, lo:lo + F], qx[:])
                else:
                    nc.vector.tensor_sub(qx[:], o2[:, lo:lo + F], qx[:])
                    nc.vector.tensor_add(qx[:], qx[:], o1[:, lo:lo + F])
                    nc.vector.tensor_add(v[:, c, :], v[:, c, :], qx[:])

            # ---- update block (PaiNN) ----
            vT = []
            for c in range(3):
                ps_vt = pp.tile([F, N], F32, tag="tr")
                nc.tensor.transpose(ps_vt[:], v[:, c, :], ident[:])
                vT_c = work.tile([F, N], F32, tag=f"vT{c}")
                nc.vector.tensor_copy(vT_c[:], ps_vt[:])
                vT.append(vT_c)
            ps_uv = pp.tile([F, 3, N], F32, tag="uv")
            ps_vv = pp.tile([F, 3, N], F32, tag="uv")
            for c in range(3):
                nc.tensor.matmul(ps_uv[:, c, :], sb["uwu"][:, l, :], vT[c][:],
                                 start=True, stop=True)
            for c in range(3):
                nc.tensor.matmul(ps_vv[:, c, :], sb["uwv"][:, l, :], vT[c][:],
                                 start=True, stop=True)

            uvs = work.tile([F, 3, N], F32, tag="uvs", bufs=1)
            nc.vector.tensor_copy(uvs[:], ps_uv[:])
            vvs = work.tile([F, 3, N], F32, tag="vvs", bufs=1)
            nc.vector.tensor_copy(vvs[:], ps_vv[:])
            vvn = work.tile([F, N], F32, tag="vvn")
            nc.vector.tensor_mul(vvn[:], vvs[:, 0, :], vvs[:, 0, :])
            dot = work.tile([F, N], F32, tag="dot")
            nc.vector.tensor_mul(dot[:], uvs[:, 0, :], vvs[:, 0, :])
            tq = work.tile([F, N], F32, tag="tq")
            for c in (1, 2):
                nc.vector.tensor_mul(tq[:], vvs[:, c, :], vvs[:, c, :])
                nc.vector.tensor_add(vvn[:], vvn[:], tq[:])
                nc.vector.tensor_mul(tq[:], uvs[:, c, :], vvs[:, c, :])
                nc.vector.tensor_add(dot[:], dot[:], tq[:])
            nc.scalar.activation(vvn[:], vvn[:], AF.Sqrt, bias=c_eps[:])

            ps3 = pp.tile([F, N], F32, tag="mm")
            nc.tensor.matmul(ps3[:], sb["uws1"][:, l, 0, :], sT[:],
                             start=True, stop=False)
            nc.tensor.matmul(ps3[:], sb["uws1"][:, l, 1, :], vvn[:],
                             start=False, stop=True)
            act2 = work.tile([F, N], F32, tag="act2")
            silu(act2[:], ps3[:], sb["ubs1"][:, l:l + 1], "z2")
            # s path first: it gates the next layer's phi matmuls
            aT = [None, None, None]
            for p in (1, 2, 0):
                ps4 = pp.tile([F, N], F32, tag="mm")
                nc.tensor.matmul(ps4[:], sb["uws2"][:, l, p * F:(p + 1) * F],
                                 act2[:], start=True, stop=True)
                aT_p = work.tile([F, N], F32, tag=f"aT{p}")
                nc.scalar.activation(aT_p[:], ps4[:], AF.Identity,
                                     bias=sb["ubs2"][:, l, p:p + 1])
                aT[p] = aT_p
                if p == 2:
                    # ds_u = a_sv * dot + a_ss ; sT += ds_uT
                    nc.vector.tensor_mul(dot[:], dot[:], aT[1][:])
                    nc.vector.tensor_add(dot[:], dot[:], aT[2][:])
                    nc.vector.tensor_add(sT[:], sT[:], dot[:])

            # dv_u = u_v * a_vv ; v += dv_u (via transpose back)
            for c in range(3):
                dvuT = work.tile([F, N], F32, tag=f"dvuT{c}")
                nc.vector.tensor_mul(dvuT[:], uvs[:, c, :], aT[0][:])
                ps_b = pp.tile([N, F], F32, tag="tr")
                nc.tensor.transpose(ps_b[:], dvuT[:], ident[:])
                nc.vector.tensor_add(v[:, c, :], v[:, c, :], ps_b[:])

        # ---- output heads ----
        def head(w1, b1, w2, b2, out_dram, is_sigma):
            psh = pp.tile([F, N], F32, tag="mm")
            nc.tensor.matmul(psh[:], sb[w1][:], sT[:], start=True, stop=True)
            th = work.tile([F, N], F32, tag="head_t")
            nc.scalar.activation(th[:], psh[:], AF.Tanh, bias=sb[b1][:])
            psh2 = pp.tile([F, N], F32, tag="mm")
            nc.tensor.matmul(psh2[:], sb[w2][:], th[:], start=True, stop=True)
            hT = work.tile([F, N], F32, tag="head_o")
            # bias-add on DVE so the transpose input's last writer is DVE
            nc.vector.tensor_scalar_add(hT[:], psh2[:], sb[b2][:])
            ps_o = pp.tile([N, F], F32, tag="tr")
            nc.tensor.transpose(ps_o[:], hT[:], ident[:])
            ho = work.tile([N, F], F32, tag="head_f")
            if is_sigma:
                # sigma = 1e-9 + exp((x + b2)/2)
                nc.scalar.activation(ho[:], ps_o[:], AF.Exp, scale=0.5)
                nc.vector.tensor_scalar_add(ho[:], ho[:], 1e-9)
            else:
                nc.vector.tensor_copy(ho[:], ps_o[:])
            nc.sync.dma_start(out_dram.ap(), ho[:])

        head("hmw1", "hmb1", "hmw2", "hmb2", out_mu, False)
        head("hsw1", "hsb1", "hsw2", "hsb2", out_sig, True)

    return nc


def _host_prep(H, cg_adj, cg_xyz, params):
    p = {k: np.asarray(v, dtype=np.float32) for k, v in params.items()}
    wtil = np.concatenate([p["rbf_W"], p["rbf_b"][:, None, :]], axis=1)
    # [L, KC, 3F] -> [L, CH, 3, KCH, F] -> replicate partitions
    nch = KC // KCH
    wtc = wtil.reshape(L, nch, KCH, 3, F).transpose(0, 1, 3, 2, 4)
    wbc = np.broadcast_to(wtc[:, :, None], (L, nch, N, 3, KCH, F))
    freqn = (np.arange(1, NRBF + 1, dtype=np.float32) * (PI / CUTOFF)
             / (2.0 * PI))
    shared = {
        "wbc": np.ascontiguousarray(wbc),
        "freqn": np.ascontiguousarray(np.tile(freqn[None, :], (N, 1))),
        "mw1": np.ascontiguousarray(p["msg_W1"].transpose(1, 0, 2)),
        "mb1": np.ascontiguousarray(p["msg_b1"].T),
        "mw2": np.ascontiguousarray(p["msg_W2"].transpose(1, 0, 2)),
        "mb2": np.ascontiguousarray(
            p["msg_b2"].reshape(L, 3, F).transpose(2, 0, 1)),
        "uwu": np.ascontiguousarray(p["upd_Wu"].transpose(1, 0, 2)),
        "uwv": np.ascontiguousarray(p["upd_Wv"].transpose(1, 0, 2)),
        "uws1": np.ascontiguousarray(
            p["upd_Ws1"].reshape(L, 2, F, F).transpose(2, 0, 1, 3)),
        "ubs1": np.ascontiguousarray(p["upd_bs1"].T),
        "uws2": np.ascontiguousarray(p["upd_Ws2"].transpose(1, 0, 2)),
        "ubs2": np.ascontiguousarray(
            p["upd_bs2"].reshape(L, 3, F).transpose(2, 0, 1)),
        "hmw1": np.ascontiguousarray(p["mu_W1"]),
        "hmb1": np.ascontiguousarray(p["mu_b1"][:, None]),
        "hmw2": np.ascontiguousarray(p["mu_W2"]),
        "hmb2": np.ascontiguousarray(p["mu_b2"][:, None]),
        "hsw1": np.ascontiguousarray(p["sig_W1"]),
        "hsb1": np.ascontiguousarray(p["sig_b1"][:, None]),
        "hsw2": np.ascontiguousarray(p["sig_W2"]),
        "hsb2": np.ascontiguousarray(p["sig_b2"][:, None]),
    }
    Ha = np.asarray(H, dtype=np.float32)
    Aa = np.asarray(cg_adj, dtype=np.float32)
    Xa = np.asarray(cg_xyz, dtype=np.float32)
    in_maps = []
    for c in range(N_CORES):
        b = c % B
        m = dict(shared)
        m["H"] = np.ascontiguousarray(Ha[b])
        m["adj"] = np.ascontiguousarray(Aa[b])
        m["xyz"] = np.ascontiguousarray(Xa[b])
        in_maps.append(m)
    return in_maps


_CACHED = {}


def _get_nc():
    if "nc" not in _CACHED:
        import concourse.bacc as bacc
        nc = bacc.Bacc("TRN2", target_bir_lowering=False, debug=False)
        build_program(nc)
        if not nc.is_finalized():
            nc.finalize()
        _CACHED["nc"] = nc
    return _CACHED["nc"]


def kernel(H, cg_adj, cg_xyz, params, _trace=False):
    from concourse.bass_utils import run_bass_kernel_spmd

    nc = _get_nc()
    in_maps = _host_prep(H, cg_adj, cg_xyz, params)
    res = run_bass_kernel_spmd(nc, in_maps, core_ids=list(range(N_CORES)),
                               trace=_trace)
    mu = np.stack([res.results[b]["out_mu"] for b in range(B)])
    sig = np.stack([res.results[b]["out_sig"] for b in range(B)])
    if _trace:
        kernel.last_exec_time_ns = res.exec_time_ns
    return (mu, sig)


if __name__ == "__main__":
    nc = _get_nc()
    print("built ok")


# revision 54
# speedup vs baseline: 1.0276x; 1.0276x over previous
"""Trainium2 Bass kernel for DenseCGPrior (PaiNN-style CG message passing).

Self-contained: hardcodes B=4, N=128, F=128, N_RBF=20, CUTOFF=5.0, L=3.
Sharding: data-parallel over batch; core c computes batch c % 4 (cores 4-7
duplicate so all 8 cores run the same SPMD program).

Key restructuring vs the reference: the [N,N,3F] per-edge message tensor is
never materialized. With 21 radial channels (20 RBF + 1 bias carrying rbf_b),
   inv[i,j,f'] = phi[j,f'] * sum_k g_k[i,j] * W[k,f']
so each edge reduction becomes 21 PSUM-accumulated matmuls with the symmetric
geometry matrices G_k (or G_k/d for the unit-vector term, which is decomposed
via unit = (x_j - x_i)/d into two matmul families plus rank-1 corrections).
"""

import os
import sys

import numpy as np

for _p in ("/opt/trn_rl_repo", "/root/.axon_site/_ro/trn_rl_repo"):
    if os.path.isdir(_p) and _p not in sys.path:
        sys.path.insert(0, _p)

import concourse.bass as bass
import concourse.mybir as mybir
import concourse.tile as tile
from concourse.masks import make_identity

F32 = mybir.dt.float32
F32R = mybir.dt.float32r
AF = mybir.ActivationFunctionType
OP = mybir.AluOpType

B, N, F, NRBF, L = 4, 128, 128, 20, 3
KC = NRBF + 1            # rbf channels + bias channel
KCH = 7                  # k-chunk size (21 = 3 chunks of 7)
F3 = 3 * F
EPS = 0.001
PI = float(np.pi)
CUTOFF = 5.0
N_CORES = 8

_IN_SPECS = [
    ("H", [N, F]),
    ("adj", [N, N]),
    ("xyz", [N, 3]),
    ("wbc", [L, 3, N, 3, KCH, F]),  # folded rbf weights, k-chunked,
                                    # replicated per partition
    ("freqn", [N, NRBF]),        # freq_k/(2pi) replicated per partition
    ("mw1", [F, L, F]),          # msg_W1 transposed to [f_in, l, f_out]
    ("mb1", [F, L]),
    ("mw2", [F, L, F3]),
    ("mb2", [F, L, 3]),          # [f, l, part]
    ("uwu", [F, L, F]),
    ("uwv", [F, L, F]),
    ("uws1", [F, L, 2, F]),      # [row_in_chunk, l, chunk, f_out]
    ("ubs1", [F, L]),
    ("uws2", [F, L, F3]),
    ("ubs2", [F, L, 3]),
    ("hmw1", [F, F]), ("hmb1", [F, 1]),
    ("hmw2", [F, F]), ("hmb2", [F, 1]),
    ("hsw1", [F, F]), ("hsb1", [F, 1]),
    ("hsw2", [F, F]), ("hsb2", [F, 1]),
]


def _rep(ap, times):
    """Read-broadcast a [P, M] AP as [P, times, M] via a step-0 free dim."""
    return bass.AP(tensor=ap.tensor, offset=ap.offset,
                   ap=[ap.ap[0], [0, times], *ap.ap[1:]])


def build_program(nc):
    dins = {name: nc.dram_tensor(name, shape, F32, kind="ExternalInput")
            for name, shape in _IN_SPECS}
    out_mu = nc.dram_tensor("out_mu", [N, F], F32, kind="ExternalOutput")
    out_sig = nc.dram_tensor("out_sig", [N, F], F32, kind="ExternalOutput")

    from contextlib import ExitStack
    with tile.TileContext(nc) as tc, ExitStack() as ctx:
        consts = ctx.enter_context(tc.tile_pool(name="consts", bufs=1))
        geom = ctx.enter_context(tc.tile_pool(name="geom", bufs=1))
        state = ctx.enter_context(tc.tile_pool(name="state", bufs=1))
        work = ctx.enter_context(tc.tile_pool(name="work", bufs=2))
        fams = ctx.enter_context(tc.tile_pool(name="fams", bufs=2))
        wbcp = ctx.enter_context(tc.tile_pool(name="wbcp", bufs=6))
        pp = ctx.enter_context(tc.tile_pool(name="pp", bufs=2, space="PSUM"))

        # ---- constants / weights to SBUF ----
        ident = consts.tile([N, N], F32)
        make_identity(nc, ident[:])
        # PE warmup on ident: absorbs the Pool-sem wait so later transposes
        # carry at most one sync wait (walrus LW struct limit).
        ps_wu = pp.tile([N, N], F32, tag="tr")
        nc.tensor.transpose(ps_wu[:], ident[:], ident[:])

        sb = {}
        _early = ("xyz", "adj", "freqn", "mw1", "mb1", "mw2", "mb2")

        def _load(names):
            for name, shape in _IN_SPECS:
                if name in ("H", "wbc") or name in sb:
                    continue
                if names is not None and name not in names:
                    continue
                t = consts.tile(shape, F32, tag=f"w_{name}")
                nc.sync.dma_start(t[:], dins[name].ap())
                sb[name] = t

        _load(_early)
        # wbc chunk 0 of layer 0 gates the first message matmuls: issue it
        # right after the small early weights, split per part so it spreads
        # across DMA queues.
        wbc0 = []
        for ch in range(KC // KCH):
            wb = wbcp.tile([N, 3, KCH, F], F32, tag="wbc")
            for p in range(3):
                nc.sync.dma_start(wb[:, p, :, :],
                                  dins["wbc"].ap()[0, ch][:, p, :, :])
            wbc0.append(wb)

        def ccol(val, tag):
            t = consts.tile([N, 1], F32, tag=tag)
            nc.vector.memset(t[:], val)
            return t

        c_eps = ccol(EPS, "c_eps")
        c_halfpi = ccol(PI / 2, "c_halfpi")

        def silu(out_t, in_ps, biascol, ztag):
            z = work.tile([F, N], F32, tag=ztag)
            nc.vector.tensor_scalar_add(z[:], in_ps, biascol)
            nc.scalar.activation(out_t, z[:], AF.Sigmoid)
            nc.vector.tensor_mul(out_t, z[:], out_t)

        # ---- geometry ----
        xyzt = sb["xyz"]
        adjt = sb["adj"]

        # xyzT [3, N] and -2*xyzT
        xyz_s = geom.tile([N, 3], F32)
        nc.vector.tensor_copy(xyz_s[:], xyzt[:])
        ps_x = pp.tile([3, N], F32, tag="tr")
        nc.tensor.transpose(ps_x[:], xyz_s[:], ident[:])
        xyzT = geom.tile([3, N], F32)
        nc.vector.tensor_copy(xyzT[:], ps_x[:])
        xyzTm2 = geom.tile([3, N], F32)
        nc.scalar.mul(xyzTm2[:], xyzT[:], -2.0)
        sqT = geom.tile([3, N], F32)
        nc.vector.tensor_mul(sqT[:], xyzT[:], xyzT[:])
        ones31 = geom.tile([3, 1], F32)
        nc.vector.memset(ones31[:], 1.0)
        ones1N = geom.tile([1, N], F32)
        nc.vector.memset(ones1N[:], 1.0)
        ps_nn = pp.tile([1, N], F32, tag="tr")
        nc.tensor.matmul(ps_nn[:], ones31[:], sqT[:], start=True, stop=True)
        nn_row = geom.tile([1, N], F32)
        nc.vector.tensor_copy(nn_row[:], ps_nn[:])

        # d2 = |xi|^2 + |xj|^2 - 2<xi,xj>  (PSUM accumulation)
        ps_d2 = pp.tile([N, N], F32, tag="tr")
        nc.tensor.matmul(ps_d2[:], xyzTm2[:], xyzT[:], start=True, stop=False)
        nc.tensor.matmul(ps_d2[:], ones1N[:], nn_row[:], start=False, stop=False)
        nc.tensor.matmul(ps_d2[:], nn_row[:], ones1N[:], start=False, stop=True)
        dmat = geom.tile([N, N], F32)
        nc.scalar.activation(dmat[:], ps_d2[:], AF.Sqrt, bias=c_eps[:])
        invd = geom.tile([N, N], F32)
        nc.vector.reciprocal(invd[:], dmat[:])

        # deg / dis / ew
        deg = geom.tile([N, 1], F32)
        nc.vector.reduce_sum(deg[:], adjt[:], axis=mybir.AxisListType.X)
        dis = geom.tile([N, 1], F32)
        nc.vector.reciprocal(dis[:], deg[:])
        nc.scalar.activation(dis[:], dis[:], AF.Sqrt, bias=c_eps[:])
        dis_s = geom.tile([N, 1], F32)
        nc.vector.tensor_copy(dis_s[:], dis[:])
        ps_dr = pp.tile([1, N], F32, tag="tr")
        nc.tensor.transpose(ps_dr[:], dis_s[:], ident[:])
        dis_row = geom.tile([1, N], F32)
        nc.vector.tensor_copy(dis_row[:], ps_dr[:])
        ps_ew = pp.tile([N, N], F32, tag="tr")
        nc.tensor.matmul(ps_ew[:], dis_row[:], dis_row[:], start=True, stop=True)
        mask = geom.tile([N, N], F32)
        nc.vector.tensor_scalar(mask[:], adjt[:], 0.0, None, op0=OP.is_gt)
        ew = geom.tile([N, N], F32)
        nc.vector.tensor_mul(ew[:], mask[:], ps_ew[:])

        # envelope: env = 0.5 + 0.5*sin(pi/2 - pi*min(d,CUTOFF)/CUTOFF)
        dc = geom.tile([N, N], F32)
        nc.vector.tensor_scalar_min(dc[:], dmat[:], CUTOFF)
        env = geom.tile([N, N], F32)
        nc.scalar.activation(env[:], dc[:], AF.Sin,
                             bias=c_halfpi[:], scale=-PI / CUTOFF)
        nc.vector.tensor_scalar(env[:], env[:], 0.5, 0.5,
                                op0=OP.mult, op1=OP.add)
        eew = geom.tile([N, N], F32)
        nc.vector.tensor_mul(eew[:], env[:], ew[:])
        sh1 = geom.tile([N, N], F32)    # env*ew/d
        nc.vector.tensor_mul(sh1[:], eew[:], invd[:])

        # G[j, k, i]: k<20 -> sin(d*freq_k)*env*ew/d ; k=20 -> env*ew
        # sin range reduction: t = d*freq_k/(2pi); frac = t - rne(t) in
        # [-1/2, 1/2]; sin(d*freq_k) = sin(2pi*frac). rne via the exact
        # float trick (t + 1.5*2^23) - 1.5*2^23, identical on DVE and numpy.
        G = geom.tile([N, KC, N], F32R)
        GD = geom.tile([N, KC, N], F32R)
        RC = 12582912.0  # 1.5 * 2^23
        fq = sb["freqn"]  # [N, NRBF] per-partition copies of freq_k/(2pi)
        fq_ap = bass.AP(tensor=fq[:].tensor, offset=fq[:].offset,
                        ap=[fq[:].ap[0], fq[:].ap[1], [0, N]])
        # Built in k-chunks of 7 so the first message matmuls can start as
        # soon as chunk 0 is ready. f32 scratches borrowed from the fam pool
        # (distinct memrefs, so the fp32r producer-rounding check is happy).
        for ch in range(KC // KCH):
            k0 = ch * KCH
            nk = min(KCH, NRBF - k0)  # last chunk: 6 rbf rows + bias row
            scr1 = fams.tile([N, KCH, N], F32, tag="fam1")
            scr2 = fams.tile([N, KCH, N], F32, tag="fam2")
            fqa = bass.AP(tensor=fq[:].tensor, offset=fq[:].offset + k0,
                          ap=[fq[:].ap[0], [1, nk], [0, N]])
            nc.vector.tensor_tensor(scr1[:, 0:nk, :], _rep(dmat[:], nk),
                                    fqa, op=OP.mult)
            nc.vector.tensor_scalar(scr2[:, 0:nk, :], scr1[:, 0:nk, :],
                                    RC, RC, op0=OP.add, op1=OP.subtract)
            nc.vector.tensor_tensor(scr1[:, 0:nk, :], scr1[:, 0:nk, :],
                                    scr2[:, 0:nk, :], op=OP.subtract)
            nc.scalar.activation(scr1[:, 0:nk, :], scr1[:, 0:nk, :],
                                 AF.Sin, scale=2.0 * PI)
            nc.vector.tensor_tensor(G[:, k0:k0 + nk, :], scr1[:, 0:nk, :],
                                    _rep(sh1[:], nk), op=OP.mult)
            if ch == KC // KCH - 1:
                nc.vector.tensor_copy(G[:, NRBF, :], eew[:])
            nc.vector.tensor_tensor(GD[:, k0:k0 + KCH, :],
                                    G[:, k0:k0 + KCH, :].bitcast(F32),
                                    _rep(invd[:], KCH), op=OP.mult)

        # ---- initial state ----
        sT = state.tile([F, N], F32)       # s transposed [f, n]
        ps_s0 = pp.tile([N, N], F32, tag="tr")
        h0 = work.tile([N, F], F32, tag="h0")
        nc.sync.dma_start(h0[:], dins["H"].ap())
        h0s = work.tile([N, F], F32, tag="h0s")
        nc.vector.tensor_copy(h0s[:], h0[:])
        nc.tensor.transpose(ps_s0[:], h0s[:], ident[:])
        nc.vector.tensor_copy(sT[:], ps_s0[:])
        v = state.tile([N, 3, F], F32)     # v[n, c, f]

        def load_wbc(l):
            """DMA layer-l folded rbf weights (host-replicated), k-chunked."""
            wbs = []
            for ch in range(KC // KCH):
                wb = wbcp.tile([N, 3, KCH, F], F32, tag="wbc")
                nc.sync.dma_start(wb[:], dins["wbc"].ap()[l, ch])
                wbs.append(wb)
            return wbs

        _load(None)  # remaining weights, lower DMA priority

        for l in range(L):
            wbc = wbc0 if l == 0 else load_wbc(l)
            # ---- phi = silu(s@W1 + b1) @ W2 + b2, produced transposed ----
            ps1 = pp.tile([F, N], F32, tag="mm")
            nc.tensor.matmul(ps1[:], sb["mw1"][:, l, :], sT[:],
                             start=True, stop=True)
            act1 = work.tile([F, N], F32, tag="act1")
            silu(act1[:], ps1[:], sb["mb1"][:, l:l + 1], "z1")
            phi = [None, None, None]
            for p in (2, 1, 0):   # part 2 first: it gates the o2 group
                ps2 = pp.tile([F, N], F32, tag="mm")
                nc.tensor.matmul(ps2[:], sb["mw2"][:, l, p * F:(p + 1) * F],
                                 act1[:], start=True, stop=True)
                phiT_p = work.tile([F, N], F32, tag=f"phiT{p}")
                nc.vector.tensor_scalar_add(phiT_p[:], ps2[:],
                                            sb["mb2"][:, l, p:p + 1])
                ps_t = pp.tile([N, F], F32, tag="tr")
                nc.tensor.transpose(ps_t[:], phiT_p[:], ident[:])
                phi_p = work.tile([N, F], F32, tag=f"phi{p}")
                nc.vector.tensor_copy(phi_p[:], ps_t[:])
                phi[p] = phi_p

            pv = []
            if l > 0:
                for c in range(3):
                    pv_c = work.tile([N, F], F32, tag=f"pv{c}")
                    nc.vector.tensor_mul(pv_c[:], phi[0][:], v[:, c, :])
                    pv.append(pv_c)

            # ---- message matmuls: 21 channels in 3 chunks of 7 ----
            nslot1 = 1 if l == 0 else 4
            o1 = pp.tile([N, nslot1 * F], F32, tag="acc")
            o2 = pp.tile([N, 4 * F], F32, tag="acc")
            for ch in range(KC // KCH):
                k0 = ch * KCH
                wb = wbc[ch]
                fam2 = fams.tile([N, KCH, 4, F], F32R, tag="fam2")
                nc.vector.tensor_tensor(fam2[:, :, 0, :],
                                        _rep(phi[2][:], KCH),
                                        wb[:, 2, :, :], op=OP.mult)
                for c in range(3):
                    nc.scalar.mul(fam2[:, :, 1 + c, :], fam2[:, :, 0, :],
                                  xyzt[:, c:c + 1])
                fam1 = fams.tile([N, KCH, nslot1, F], F32R, tag="fam1")
                nc.gpsimd.tensor_tensor(fam1[:, :, 0, :],
                                        _rep(phi[1][:], KCH),
                                        wb[:, 1, :, :], op=OP.mult)
                for c in range(3) if l > 0 else ():
                    nc.vector.tensor_tensor(fam1[:, :, 1 + c, :],
                                            _rep(pv[c][:], KCH),
                                            wb[:, 0, :, :],
                                            op=OP.mult)
                for kk in range(KCH):
                    k = k0 + kk
                    nc.tensor.matmul(
                        o2[:], GD[:, k, :],
                        fam2[:, kk, :, :].rearrange("p s f -> p (s f)"),
                        start=(k == 0), stop=(k == KC - 1))
                for kk in range(KCH):
                    k = k0 + kk
                    nc.tensor.matmul(
                        o1[:], G[:, k, :],
                        fam1[:, kk, :, :].rearrange("p s f -> p (s f)"),
                        start=(k == 0), stop=(k == KC - 1))

            # ---- apply ds / dv ----
            ds = work.tile([N, F], F32, tag="ds")
            nc.vector.tensor_copy(ds[:], o1[:, 0:F])
            ps_dst = pp.tile([F, N], F32, tag="tr")
            nc.tensor.transpose(ps_dst[:], ds[:], ident[:])
            nc.vector.tensor_add(sT[:], sT[:], ps_dst[:])

            for c in range(3):
                qx = work.tile([N, F], F32, tag=f"qx{c}")
                nc.vector.tensor_scalar_mul(qx[:], o2[:, 0:F],
                                            xyzt[:, c:c + 1])
                lo = (1 + c) * F
                if l == 0:
                    nc.vector.tensor_sub(v[:, c, :], o2[:, lo:lo + F], qx[:])
                else:
                    nc.vector.tensor_sub(qx[:], o2[:, lo:lo + F], qx[:])
                    nc.vector.tensor_add(qx[:], qx[:], o1[:, lo:lo + F])
                    nc.vector.tensor_add(v[:, c, :], v[:, c, :], qx[:])

            # ---- update block (PaiNN) ----
            vT = []
            for c in range(3):
                ps_vt = pp.tile([F, N], F32, tag="tr")
                nc.tensor.transpose(ps_vt[:], v[:, c, :], ident[:])
                vT_c = work.tile([F, N], F32, tag=f"vT{c}")
                nc.vector.tensor_copy(vT_c[:], ps_vt[:])
                vT.append(vT_c)
            ps_uv = pp.tile([F, 3, N], F32, tag="uv")
            ps_vv = pp.tile([F, 3, N], F32, tag="uv")
            for c in range(3):
                nc.tensor.matmul(ps_uv[:, c, :], sb["uwu"][:, l, :], vT[c][:],
                                 start=True, stop=True)
            for c in range(3):
                nc.tensor.matmul(ps_vv[:, c, :], sb["uwv"][:, l, :], vT[c][:],
                                 start=True, stop=True)

            uvs = work.tile([F, 3, N], F32, tag="uvs", bufs=1)
            nc.vector.tensor_copy(uvs[:], ps_uv[:])
            vvs = work.tile([F, 3, N], F32, tag="vvs", bufs=1)
            nc.vector.tensor_copy(vvs[:], ps_vv[:])
            vvn = work.tile([F, N], F32, tag="vvn")
            nc.vector.tensor_mul(vvn[:], vvs[:, 0, :], vvs[:, 0, :])
            dot = work.tile([F, N], F32, tag="dot")
            nc.vector.tensor_mul(dot[:], uvs[:, 0, :], vvs[:, 0, :])
            tq = work.tile([F, N], F32, tag="tq")
            for c in (1, 2):
                nc.vector.tensor_mul(tq[:], vvs[:, c, :], vvs[:, c, :])
                nc.vector.tensor_add(vvn[:], vvn[:], tq[:])
                nc.vector.tensor_mul(tq[:], uvs[:, c, :], vvs[:, c, :])
                nc.vector.tensor_add(dot[:], dot[:], tq[:])
            nc.scalar.activation(vvn[:], vvn[:], AF.Sqrt, bias=c_eps[:])
            # dep-pinned PE filler: keeps the HAM activity window alive
            # through this serial DVE/ACT stretch (output never read)
            ps_f1 = pp.tile([N, F], F32, tag="tr")
            nc.tensor.transpose(ps_f1[:], dot[:], ident[:])

            ps3 = pp.tile([F, N], F32, tag="mm")
            nc.tensor.matmul(ps3[:], sb["uws1"][:, l, 0, :], sT[:],
                             start=True, stop=False)
            nc.tensor.matmul(ps3[:], sb["uws1"][:, l, 1, :], vvn[:],
                             start=False, stop=True)
            act2 = work.tile([F, N], F32, tag="act2")
            silu(act2[:], ps3[:], sb["ubs1"][:, l:l + 1], "z2")
            ps_f2 = pp.tile([N, F], F32, tag="tr")
            nc.tensor.transpose(ps_f2[:], act2[:], ident[:])
            # s path first: it gates the next layer's phi matmuls
            aT = [None, None, None]
            for p in (1, 2, 0):
                ps4 = pp.tile([F, N], F32, tag="mm")
                nc.tensor.matmul(ps4[:], sb["uws2"][:, l, p * F:(p + 1) * F],
                                 act2[:], start=True, stop=True)
                aT_p = work.tile([F, N], F32, tag=f"aT{p}")
                nc.scalar.activation(aT_p[:], ps4[:], AF.Identity,
                                     bias=sb["ubs2"][:, l, p:p + 1])
                aT[p] = aT_p
                if p == 2:
                    # ds_u = a_sv * dot + a_ss ; sT += ds_uT
                    nc.vector.tensor_mul(dot[:], dot[:], aT[1][:])
                    nc.vector.tensor_add(dot[:], dot[:], aT[2][:])
                    nc.vector.tensor_add(sT[:], sT[:], dot[:])

            # dv_u = u_v * a_vv ; v += dv_u (via transpose back)
            for c in range(3):
                dvuT = work.tile([F, N], F32, tag=f"dvuT{c}")
                nc.vector.tensor_mul(dvuT[:], uvs[:, c, :], aT[0][:])
                ps_b = pp.tile([N, F], F32, tag="tr")
                nc.tensor.transpose(ps_b[:], dvuT[:], ident[:])
                nc.vector.tensor_add(v[:, c, :], v[:, c, :], ps_b[:])

        # ---- output heads ----
        def head(w1, b1, w2, b2, out_dram, is_sigma):
            psh = pp.tile([F, N], F32, tag="mm")
            nc.tensor.matmul(psh[:], sb[w1][:], sT[:], start=True, stop=True)
            th = work.tile([F, N], F32, tag="head_t")
            nc.scalar.activation(th[:], psh[:], AF.Tanh, bias=sb[b1][:])
            psh2 = pp.tile([F, N], F32, tag="mm")
            nc.tensor.matmul(psh2[:], sb[w2][:], th[:], start=True, stop=True)
            hT = work.tile([F, N], F32, tag="head_o")
            # bias-add on DVE so the transpose input's last writer is DVE
            nc.vector.tensor_scalar_add(hT[:], psh2[:], sb[b2][:])
            ps_o = pp.tile([N, F], F32, tag="tr")
            nc.tensor.transpose(ps_o[:], hT[:], ident[:])
            ho = work.tile([N, F], F32, tag="head_f")
            if is_sigma:
                # sigma = 1e-9 + exp((x + b2)/2)
                nc.scalar.activation(ho[:], ps_o[:], AF.Exp, scale=0.5)
                nc.vector.tensor_scalar_add(ho[:], ho[:], 1e-9)
            else:
                nc.vector.tensor_copy(ho[:], ps_o[:])
            nc.sync.dma_start(out_dram.ap(), ho[:])

        head("hmw1", "hmb1", "hmw2", "hmb2", out_mu, False)
        head("hsw1", "hsb1", "hsw2", "hsb2", out_sig, True)

    return nc


def _host_prep(H, cg_adj, cg_xyz, params):
    p = {k: np.asarray(v, dtype=np.float32) for k, v in params.items()}
    wtil = np.concatenate([p["rbf_W"], p["rbf_b"][:, None, :]], axis=1)
    # [L, KC, 3F] -> [L, CH, 3, KCH, F] -> replicate partitions
    nch = KC // KCH
    wtc = wtil.reshape(L, nch, KCH, 3, F).transpose(0, 1, 3, 2, 4)
    wbc = np.broadcast_to(wtc[:, :, None], (L, nch, N, 3, KCH, F))
    freqn = (np.arange(1, NRBF + 1, dtype=np.float32) * (PI / CUTOFF)
             / (2.0 * PI))
    shared = {
        "wbc": np.ascontiguousarray(wbc),
        "freqn": np.ascontiguousarray(np.tile(freqn[None, :], (N, 1))),
        "mw1": np.ascontiguousarray(p["msg_W1"].transpose(1, 0, 2)),
        "mb1": np.ascontiguousarray(p["msg_b1"].T),
        "mw2": np.ascontiguousarray(p["msg_W2"].transpose(1, 0, 2)),
        "mb2": np.ascontiguousarray(
            p["msg_b2"].reshape(L, 3, F).transpose(2, 0, 1)),
        "uwu": np.ascontiguousarray(p["upd_Wu"].transpose(1, 0, 2)),
        "uwv": np.ascontiguousarray(p["upd_Wv"].transpose(1, 0, 2)),
        "uws1": np.ascontiguousarray(
            p["upd_Ws1"].reshape(L, 2, F, F).transpose(2, 0, 1, 3)),
        "ubs1": np.ascontiguousarray(p["upd_bs1"].T),
        "uws2": np.ascontiguousarray(p["upd_Ws2"].transpose(1, 0, 2)),
        "ubs2": np.ascontiguousarray(
            p["upd_bs2"].reshape(L, 3, F).transpose(2, 0, 1)),
        "hmw1": np.ascontiguousarray(p["mu_W1"]),
        "hmb1": np.ascontiguousarray(p["mu_b1"][:, None]),
        "hmw2": np.ascontiguousarray(p["mu_W2"]),
        "hmb2": np.ascontiguousarray(p["mu_b2"][:, None]),
        "hsw1": np.ascontiguousarray(p["sig_W1"]),
        "hsb1": np.ascontiguousarray(p["sig_b1"][:, None]),
        "hsw2": np.ascontiguousarray(p["sig_W2"]),
        "hsb2": np.ascontiguousarray(p["sig_b2"][:, None]),
    }
    Ha = np.asarray(H, dtype=np.float32)
    Aa = np.asarray(cg_adj, dtype=np.float32)
    Xa = np.asarray(cg_xyz, dtype=np.float32)
    in_maps = []
    for c in range(N_CORES):
        b = c % B
        m = dict(shared)
        m["H"] = np.ascontiguousarray(Ha[b])
        m["adj"] = np.ascontiguousarray(Aa[b])
        m["xyz"] = np.ascontiguousarray(Xa[b])
        in_maps.append(m)
    return in_maps


_CACHED = {}


def _get_nc():
    if "nc" not in _CACHED:
        import concourse.bacc as bacc
        nc = bacc.Bacc("TRN2", target_bir_lowering=False, debug=False)
        build_program(nc)
        if not nc.is_finalized():
            nc.finalize()
        _CACHED["nc"] = nc
    return _CACHED["nc"]


def kernel(H, cg_adj, cg_xyz, params, _trace=False):
    from concourse.bass_utils import run_bass_kernel_spmd

    nc = _get_nc()
    in_maps = _host_prep(H, cg_adj, cg_xyz, params)
    res = run_bass_kernel_spmd(nc, in_maps, core_ids=list(range(N_CORES)),
                               trace=_trace)
    mu = np.stack([res.results[b]["out_mu"] for b in range(B)])
    sig = np.stack([res.results[b]["out_sig"] for b in range(B)])
    if _trace:
        kernel.last_exec_time_ns = res.exec_time_ns
    return (mu, sig)


if __name__ == "__main__":
    nc = _get_nc()
    print("built ok")


# revision 58
# speedup vs baseline: 1.0531x; 1.0248x over previous
"""Trainium2 Bass kernel for DenseCGPrior (PaiNN-style CG message passing).

Self-contained: hardcodes B=4, N=128, F=128, N_RBF=20, CUTOFF=5.0, L=3.
Sharding: data-parallel over batch; core c computes batch c % 4 (cores 4-7
duplicate so all 8 cores run the same SPMD program).

Key restructuring vs the reference: the [N,N,3F] per-edge message tensor is
never materialized. With 21 radial channels (20 RBF + 1 bias carrying rbf_b),
   inv[i,j,f'] = phi[j,f'] * sum_k g_k[i,j] * W[k,f']
so each edge reduction becomes 21 PSUM-accumulated matmuls with the symmetric
geometry matrices G_k (or G_k/d for the unit-vector term, which is decomposed
via unit = (x_j - x_i)/d into two matmul families plus rank-1 corrections).
"""

import os
import sys

import numpy as np

for _p in ("/opt/trn_rl_repo", "/root/.axon_site/_ro/trn_rl_repo"):
    if os.path.isdir(_p) and _p not in sys.path:
        sys.path.insert(0, _p)

import concourse.bass as bass
import concourse.mybir as mybir
import concourse.tile as tile
from concourse.masks import make_identity

F32 = mybir.dt.float32
F32R = mybir.dt.float32r
AF = mybir.ActivationFunctionType
OP = mybir.AluOpType

B, N, F, NRBF, L = 4, 128, 128, 20, 3
KC = NRBF + 1            # rbf channels + bias channel
KCH = 7                  # k-chunk size (21 = 3 chunks of 7)
F3 = 3 * F
EPS = 0.001
PI = float(np.pi)
CUTOFF = 5.0
N_CORES = 8

_IN_SPECS = [
    ("H", [N, F]),
    ("adj", [N, N]),
    ("xyz", [N, 3]),
    ("wbc", [L, 3, N, 3, KCH, F]),  # folded rbf weights, k-chunked,
                                    # replicated per partition
    ("freqn", [N, NRBF]),        # freq_k/(2pi) replicated per partition
    ("mw1", [F, L, F]),          # msg_W1 transposed to [f_in, l, f_out]
    ("mb1", [F, L]),
    ("mw2", [F, L, F3]),
    ("mb2", [F, L, 3]),          # [f, l, part]
    ("uwu", [F, L, F]),
    ("uwv", [F, L, F]),
    ("uws1", [F, L, 2, F]),      # [row_in_chunk, l, chunk, f_out]
    ("ubs1", [F, L]),
    ("uws2", [F, L, F3]),
    ("ubs2", [F, L, 3]),
    ("hmw1", [F, F]), ("hmb1", [F, 1]),
    ("hmw2", [F, F]), ("hmb2", [F, 1]),
    ("hsw1", [F, F]), ("hsb1", [F, 1]),
    ("hsw2", [F, F]), ("hsb2", [F, 1]),
]


def _rep(ap, times):
    """Read-broadcast a [P, M] AP as [P, times, M] via a step-0 free dim."""
    return bass.AP(tensor=ap.tensor, offset=ap.offset,
                   ap=[ap.ap[0], [0, times], *ap.ap[1:]])


def build_program(nc):
    dins = {name: nc.dram_tensor(name, shape, F32, kind="ExternalInput")
            for name, shape in _IN_SPECS}
    out_mu = nc.dram_tensor("out_mu", [N, F], F32, kind="ExternalOutput")
    out_sig = nc.dram_tensor("out_sig", [N, F], F32, kind="ExternalOutput")

    from contextlib import ExitStack
    with tile.TileContext(nc) as tc, ExitStack() as ctx:
        consts = ctx.enter_context(tc.tile_pool(name="consts", bufs=1))
        geom = ctx.enter_context(tc.tile_pool(name="geom", bufs=1))
        state = ctx.enter_context(tc.tile_pool(name="state", bufs=1))
        work = ctx.enter_context(tc.tile_pool(name="work", bufs=2))
        fams = ctx.enter_context(tc.tile_pool(name="fams", bufs=2))
        wbcp = ctx.enter_context(tc.tile_pool(name="wbcp", bufs=18))
        pp = ctx.enter_context(tc.tile_pool(name="pp", bufs=2, space="PSUM"))

        # ---- constants / weights to SBUF ----
        ident = consts.tile([N, N], F32)
        make_identity(nc, ident[:])
        # PE warmup on ident: absorbs the Pool-sem wait so later transposes
        # carry at most one sync wait (walrus LW struct limit).
        ps_wu = pp.tile([N, N], F32, tag="tr")
        nc.tensor.transpose(ps_wu[:], ident[:], ident[:])

        sb = {}
        _early = ("xyz", "adj", "freqn", "mw1", "mb1", "mw2", "mb2")

        def _load(names):
            for name, shape in _IN_SPECS:
                if name in ("H", "wbc") or name in sb:
                    continue
                if names is not None and name not in names:
                    continue
                t = consts.tile(shape, F32, tag=f"w_{name}")
                nc.sync.dma_start(t[:], dins[name].ap())
                sb[name] = t

        def load_wbc(l):
            """DMA layer-l folded rbf weights; one tile per (chunk, part),
            issued part-2 first (part 2 gates the o2 group, part 1 the o1
            slot-0 fold, part 0 only the l>0 dv1 folds)."""
            wbs = [[None] * 3 for _ in range(KC // KCH)]
            for p in (2, 1, 0):
                for ch in range(KC // KCH):
                    wb = wbcp.tile([N, KCH, F], F32, tag="wbc")
                    nc.sync.dma_start(wb[:],
                                      dins["wbc"].ap()[l, ch][:, p, :, :])
                    wbs[ch][p] = wb
            return wbs

        _load(_early)
        # wbc of layer 0 gates the first message matmuls: issue right after
        # the small early weights.
        wbc0 = load_wbc(0)

        def ccol(val, tag):
            t = consts.tile([N, 1], F32, tag=tag)
            nc.vector.memset(t[:], val)
            return t

        c_eps = ccol(EPS, "c_eps")
        c_halfpi = ccol(PI / 2, "c_halfpi")

        def silu(out_t, in_ps, biascol, ztag):
            z = work.tile([F, N], F32, tag=ztag)
            nc.vector.tensor_scalar_add(z[:], in_ps, biascol)
            nc.scalar.activation(out_t, z[:], AF.Sigmoid)
            nc.vector.tensor_mul(out_t, z[:], out_t)

        # ---- geometry ----
        xyzt = sb["xyz"]
        adjt = sb["adj"]

        # xyzT [3, N] and -2*xyzT
        xyz_s = geom.tile([N, 3], F32)
        nc.vector.tensor_copy(xyz_s[:], xyzt[:])
        ps_x = pp.tile([3, N], F32, tag="tr")
        nc.tensor.transpose(ps_x[:], xyz_s[:], ident[:])
        xyzT = geom.tile([3, N], F32)
        nc.vector.tensor_copy(xyzT[:], ps_x[:])
        xyzTm2 = geom.tile([3, N], F32)
        nc.scalar.mul(xyzTm2[:], xyzT[:], -2.0)
        sqT = geom.tile([3, N], F32)
        nc.vector.tensor_mul(sqT[:], xyzT[:], xyzT[:])
        ones31 = geom.tile([3, 1], F32)
        nc.vector.memset(ones31[:], 1.0)
        ones1N = geom.tile([1, N], F32)
        nc.vector.memset(ones1N[:], 1.0)
        ps_nn = pp.tile([1, N], F32, tag="tr")
        nc.tensor.matmul(ps_nn[:], ones31[:], sqT[:], start=True, stop=True)
        nn_row = geom.tile([1, N], F32)
        nc.vector.tensor_copy(nn_row[:], ps_nn[:])

        # d2 = |xi|^2 + |xj|^2 - 2<xi,xj>  (PSUM accumulation)
        ps_d2 = pp.tile([N, N], F32, tag="tr")
        nc.tensor.matmul(ps_d2[:], xyzTm2[:], xyzT[:], start=True, stop=False)
        nc.tensor.matmul(ps_d2[:], ones1N[:], nn_row[:], start=False, stop=False)
        nc.tensor.matmul(ps_d2[:], nn_row[:], ones1N[:], start=False, stop=True)
        dmat = geom.tile([N, N], F32)
        nc.scalar.activation(dmat[:], ps_d2[:], AF.Sqrt, bias=c_eps[:])
        invd = geom.tile([N, N], F32)
        nc.vector.reciprocal(invd[:], dmat[:])

        # deg / dis / ew
        deg = geom.tile([N, 1], F32)
        nc.vector.reduce_sum(deg[:], adjt[:], axis=mybir.AxisListType.X)
        dis = geom.tile([N, 1], F32)
        nc.vector.reciprocal(dis[:], deg[:])
        nc.scalar.activation(dis[:], dis[:], AF.Sqrt, bias=c_eps[:])
        dis_s = geom.tile([N, 1], F32)
        nc.vector.tensor_copy(dis_s[:], dis[:])
        ps_dr = pp.tile([1, N], F32, tag="tr")
        nc.tensor.transpose(ps_dr[:], dis_s[:], ident[:])
        dis_row = geom.tile([1, N], F32)
        nc.vector.tensor_copy(dis_row[:], ps_dr[:])
        ps_ew = pp.tile([N, N], F32, tag="tr")
        nc.tensor.matmul(ps_ew[:], dis_row[:], dis_row[:], start=True, stop=True)
        mask = geom.tile([N, N], F32)
        nc.vector.tensor_scalar(mask[:], adjt[:], 0.0, None, op0=OP.is_gt)
        ew = geom.tile([N, N], F32)
        nc.vector.tensor_mul(ew[:], mask[:], ps_ew[:])

        # envelope: env = 0.5 + 0.5*sin(pi/2 - pi*min(d,CUTOFF)/CUTOFF)
        dc = geom.tile([N, N], F32)
        nc.vector.tensor_scalar_min(dc[:], dmat[:], CUTOFF)
        env = geom.tile([N, N], F32)
        nc.scalar.activation(env[:], dc[:], AF.Sin,
                             bias=c_halfpi[:], scale=-PI / CUTOFF)
        nc.vector.tensor_scalar(env[:], env[:], 0.5, 0.5,
                                op0=OP.mult, op1=OP.add)
        eew = geom.tile([N, N], F32)
        nc.vector.tensor_mul(eew[:], env[:], ew[:])
        sh1 = geom.tile([N, N], F32)    # env*ew/d
        nc.vector.tensor_mul(sh1[:], eew[:], invd[:])

        # G[j, k, i]: k<20 -> sin(d*freq_k)*env*ew/d ; k=20 -> env*ew
        # sin range reduction: t = d*freq_k/(2pi); frac = t - rne(t) in
        # [-1/2, 1/2]; sin(d*freq_k) = sin(2pi*frac). rne via the exact
        # float trick (t + 1.5*2^23) - 1.5*2^23, identical on DVE and numpy.
        G = geom.tile([N, KC, N], F32R)
        GD = geom.tile([N, KC, N], F32R)
        RC = 12582912.0  # 1.5 * 2^23
        fq = sb["freqn"]  # [N, NRBF] per-partition copies of freq_k/(2pi)
        fq_ap = bass.AP(tensor=fq[:].tensor, offset=fq[:].offset,
                        ap=[fq[:].ap[0], fq[:].ap[1], [0, N]])
        # Built in k-chunks of 7 so the first message matmuls can start as
        # soon as chunk 0 is ready. f32 scratches borrowed from the fam pool
        # (distinct memrefs, so the fp32r producer-rounding check is happy).
        for ch in range(KC // KCH):
            k0 = ch * KCH
            nk = min(KCH, NRBF - k0)  # last chunk: 6 rbf rows + bias row
            scr1 = fams.tile([N, KCH, N], F32, tag="fam1")
            scr2 = fams.tile([N, KCH, N], F32, tag="fam2")
            fqa = bass.AP(tensor=fq[:].tensor, offset=fq[:].offset + k0,
                          ap=[fq[:].ap[0], [1, nk], [0, N]])
            nc.vector.tensor_tensor(scr1[:, 0:nk, :], _rep(dmat[:], nk),
                                    fqa, op=OP.mult)
            nc.vector.tensor_scalar(scr2[:, 0:nk, :], scr1[:, 0:nk, :],
                                    RC, RC, op0=OP.add, op1=OP.subtract)
            nc.vector.tensor_tensor(scr1[:, 0:nk, :], scr1[:, 0:nk, :],
                                    scr2[:, 0:nk, :], op=OP.subtract)
            nc.scalar.activation(scr1[:, 0:nk, :], scr1[:, 0:nk, :],
                                 AF.Sin, scale=2.0 * PI)
            nc.vector.tensor_tensor(G[:, k0:k0 + nk, :], scr1[:, 0:nk, :],
                                    _rep(sh1[:], nk), op=OP.mult)
            if ch == KC // KCH - 1:
                nc.vector.tensor_copy(G[:, NRBF, :], eew[:])
            nc.vector.tensor_tensor(GD[:, k0:k0 + KCH, :],
                                    G[:, k0:k0 + KCH, :].bitcast(F32),
                                    _rep(invd[:], KCH), op=OP.mult)

        # ---- initial state ----
        sT = state.tile([F, N], F32)       # s transposed [f, n]
        ps_s0 = pp.tile([N, N], F32, tag="tr")
        h0 = work.tile([N, F], F32, tag="h0")
        nc.sync.dma_start(h0[:], dins["H"].ap())
        h0s = work.tile([N, F], F32, tag="h0s")
        nc.vector.tensor_copy(h0s[:], h0[:])
        nc.tensor.transpose(ps_s0[:], h0s[:], ident[:])
        nc.vector.tensor_copy(sT[:], ps_s0[:])
        v = state.tile([N, 3, F], F32)     # v[n, c, f]

        _load(None)  # remaining weights, lower DMA priority

        for l in range(L):
            wbc = wbc0 if l == 0 else load_wbc(l)
            # ---- phi = silu(s@W1 + b1) @ W2 + b2, produced transposed ----
            ps1 = pp.tile([F, N], F32, tag="mm")
            nc.tensor.matmul(ps1[:], sb["mw1"][:, l, :], sT[:],
                             start=True, stop=True)
            act1 = work.tile([F, N], F32, tag="act1")
            silu(act1[:], ps1[:], sb["mb1"][:, l:l + 1], "z1")
            phi = [None, None, None]
            for p in (2, 1, 0):   # part 2 first: it gates the o2 group
                ps2 = pp.tile([F, N], F32, tag="mm")
                nc.tensor.matmul(ps2[:], sb["mw2"][:, l, p * F:(p + 1) * F],
                                 act1[:], start=True, stop=True)
                phiT_p = work.tile([F, N], F32, tag=f"phiT{p}")
                nc.vector.tensor_scalar_add(phiT_p[:], ps2[:],
                                            sb["mb2"][:, l, p:p + 1])
                ps_t = pp.tile([N, F], F32, tag="tr")
                nc.tensor.transpose(ps_t[:], phiT_p[:], ident[:])
                phi_p = work.tile([N, F], F32, tag=f"phi{p}")
                nc.vector.tensor_copy(phi_p[:], ps_t[:])
                phi[p] = phi_p

            pv = []
            if l > 0:
                for c in range(3):
                    pv_c = work.tile([N, F], F32, tag=f"pv{c}")
                    nc.vector.tensor_mul(pv_c[:], phi[0][:], v[:, c, :])
                    pv.append(pv_c)

            # ---- message matmuls: 21 channels in 3 chunks of 7 ----
            nslot1 = 1 if l == 0 else 4
            o1 = pp.tile([N, nslot1 * F], F32, tag="acc")
            o2 = pp.tile([N, 4 * F], F32, tag="acc")
            for ch in range(KC // KCH):
                k0 = ch * KCH
                fam2 = fams.tile([N, KCH, 4, F], F32R, tag="fam2")
                nc.vector.tensor_tensor(fam2[:, :, 0, :],
                                        _rep(phi[2][:], KCH),
                                        wbc[ch][2][:], op=OP.mult)
                for c in range(3):
                    nc.scalar.mul(fam2[:, :, 1 + c, :], fam2[:, :, 0, :],
                                  xyzt[:, c:c + 1])
                fam1 = fams.tile([N, KCH, nslot1, F], F32R, tag="fam1")
                nc.gpsimd.tensor_tensor(fam1[:, :, 0, :],
                                        _rep(phi[1][:], KCH),
                                        wbc[ch][1][:], op=OP.mult)
                for c in range(3) if l > 0 else ():
                    nc.vector.tensor_tensor(fam1[:, :, 1 + c, :],
                                            _rep(pv[c][:], KCH),
                                            wbc[ch][0][:],
                                            op=OP.mult)
                for kk in range(KCH):
                    k = k0 + kk
                    nc.tensor.matmul(
                        o2[:], GD[:, k, :],
                        fam2[:, kk, :, :].rearrange("p s f -> p (s f)"),
                        start=(k == 0), stop=(k == KC - 1))
                for kk in range(KCH):
                    k = k0 + kk
                    nc.tensor.matmul(
                        o1[:], G[:, k, :],
                        fam1[:, kk, :, :].rearrange("p s f -> p (s f)"),
                        start=(k == 0), stop=(k == KC - 1))

            # ---- apply ds / dv ----
            ds = work.tile([N, F], F32, tag="ds")
            nc.vector.tensor_copy(ds[:], o1[:, 0:F])
            ps_dst = pp.tile([F, N], F32, tag="tr")
            nc.tensor.transpose(ps_dst[:], ds[:], ident[:])
            nc.vector.tensor_add(sT[:], sT[:], ps_dst[:])

            for c in range(3):
                qx = work.tile([N, F], F32, tag=f"qx{c}")
                nc.vector.tensor_scalar_mul(qx[:], o2[:, 0:F],
                                            xyzt[:, c:c + 1])
                lo = (1 + c) * F
                if l == 0:
                    nc.vector.tensor_sub(v[:, c, :], o2[:, lo:lo + F], qx[:])
                else:
                    nc.vector.tensor_sub(qx[:], o2[:, lo:lo + F], qx[:])
                    nc.vector.tensor_add(qx[:], qx[:], o1[:, lo:lo + F])
                    nc.vector.tensor_add(v[:, c, :], v[:, c, :], qx[:])

            # ---- update block (PaiNN) ----
            vT = []
            for c in range(3):
                ps_vt = pp.tile([F, N], F32, tag="tr")
                nc.tensor.transpose(ps_vt[:], v[:, c, :], ident[:])
                vT_c = work.tile([F, N], F32, tag=f"vT{c}")
                nc.vector.tensor_copy(vT_c[:], ps_vt[:])
                vT.append(vT_c)
            ps_uv = pp.tile([F, 3, N], F32, tag="uv")
            ps_vv = pp.tile([F, 3, N], F32, tag="uv")
            for c in range(3):
                nc.tensor.matmul(ps_uv[:, c, :], sb["uwu"][:, l, :], vT[c][:],
                                 start=True, stop=True)
            for c in range(3):
                nc.tensor.matmul(ps_vv[:, c, :], sb["uwv"][:, l, :], vT[c][:],
                                 start=True, stop=True)

            uvs = work.tile([F, 3, N], F32, tag="uvs", bufs=1)
            nc.vector.tensor_copy(uvs[:], ps_uv[:])
            vvs = work.tile([F, 3, N], F32, tag="vvs", bufs=1)
            nc.vector.tensor_copy(vvs[:], ps_vv[:])
            vvn = work.tile([F, N], F32, tag="vvn")
            nc.vector.tensor_mul(vvn[:], vvs[:, 0, :], vvs[:, 0, :])
            dot = work.tile([F, N], F32, tag="dot")
            nc.vector.tensor_mul(dot[:], uvs[:, 0, :], vvs[:, 0, :])
            tq = work.tile([F, N], F32, tag="tq")
            for c in (1, 2):
                nc.vector.tensor_mul(tq[:], vvs[:, c, :], vvs[:, c, :])
                nc.vector.tensor_add(vvn[:], vvn[:], tq[:])
                nc.vector.tensor_mul(tq[:], uvs[:, c, :], vvs[:, c, :])
                nc.vector.tensor_add(dot[:], dot[:], tq[:])
            nc.scalar.activation(vvn[:], vvn[:], AF.Sqrt, bias=c_eps[:])
            # dep-pinned PE filler: keeps the HAM activity window alive
            # through this serial DVE/ACT stretch (output never read)
            ps_f1 = pp.tile([N, F], F32, tag="tr")
            nc.tensor.transpose(ps_f1[:], dot[:], ident[:])

            ps3 = pp.tile([F, N], F32, tag="mm")
            nc.tensor.matmul(ps3[:], sb["uws1"][:, l, 0, :], sT[:],
                             start=True, stop=False)
            nc.tensor.matmul(ps3[:], sb["uws1"][:, l, 1, :], vvn[:],
                             start=False, stop=True)
            act2 = work.tile([F, N], F32, tag="act2")
            silu(act2[:], ps3[:], sb["ubs1"][:, l:l + 1], "z2")
            ps_f2 = pp.tile([N, F], F32, tag="tr")
            nc.tensor.transpose(ps_f2[:], act2[:], ident[:])
            # s path first: it gates the next layer's phi matmuls
            aT = [None, None, None]
            for p in (1, 2, 0):
                ps4 = pp.tile([F, N], F32, tag="mm")
                nc.tensor.matmul(ps4[:], sb["uws2"][:, l, p * F:(p + 1) * F],
                                 act2[:], start=True, stop=True)
                aT_p = work.tile([F, N], F32, tag=f"aT{p}")
                nc.scalar.activation(aT_p[:], ps4[:], AF.Identity,
                                     bias=sb["ubs2"][:, l, p:p + 1])
                aT[p] = aT_p
                if p == 2:
                    # ds_u = a_sv * dot + a_ss ; sT += ds_uT
                    nc.vector.tensor_mul(dot[:], dot[:], aT[1][:])
                    nc.vector.tensor_add(dot[:], dot[:], aT[2][:])
                    nc.vector.tensor_add(sT[:], sT[:], dot[:])

            # dv_u = u_v * a_vv ; v += dv_u (via transpose back)
            for c in range(3):
                dvuT = work.tile([F, N], F32, tag=f"dvuT{c}")
                nc.vector.tensor_mul(dvuT[:], uvs[:, c, :], aT[0][:])
                ps_b = pp.tile([N, F], F32, tag="tr")
                nc.tensor.transpose(ps_b[:], dvuT[:], ident[:])
                nc.vector.tensor_add(v[:, c, :], v[:, c, :], ps_b[:])

        # ---- output heads ----
        def head(w1, b1, w2, b2, out_dram, is_sigma):
            psh = pp.tile([F, N], F32, tag="mm")
            nc.tensor.matmul(psh[:], sb[w1][:], sT[:], start=True, stop=True)
            th = work.tile([F, N], F32, tag="head_t")
            nc.scalar.activation(th[:], psh[:], AF.Tanh, bias=sb[b1][:])
            psh2 = pp.tile([F, N], F32, tag="mm")
            nc.tensor.matmul(psh2[:], sb[w2][:], th[:], start=True, stop=True)
            hT = work.tile([F, N], F32, tag="head_o")
            # bias-add on DVE so the transpose input's last writer is DVE
            nc.vector.tensor_scalar_add(hT[:], psh2[:], sb[b2][:])
            ps_o = pp.tile([N, F], F32, tag="tr")
            nc.tensor.transpose(ps_o[:], hT[:], ident[:])
            ho = work.tile([N, F], F32, tag="head_f")
            if is_sigma:
                # sigma = 1e-9 + exp((x + b2)/2)
                nc.scalar.activation(ho[:], ps_o[:], AF.Exp, scale=0.5)
                nc.vector.tensor_scalar_add(ho[:], ho[:], 1e-9)
            else:
                nc.vector.tensor_copy(ho[:], ps_o[:])
            nc.sync.dma_start(out_dram.ap(), ho[:])

        head("hmw1", "hmb1", "hmw2", "hmb2", out_mu, False)
        head("hsw1", "hsb1", "hsw2", "hsb2", out_sig, True)

    return nc


def _host_prep(H, cg_adj, cg_xyz, params):
    p = {k: np.asarray(v, dtype=np.float32) for k, v in params.items()}
    wtil = np.concatenate([p["rbf_W"], p["rbf_b"][:, None, :]], axis=1)
    # [L, KC, 3F] -> [L, CH, 3, KCH, F] -> replicate partitions
    nch = KC // KCH
    wtc = wtil.reshape(L, nch, KCH, 3, F).transpose(0, 1, 3, 2, 4)
    wbc = np.broadcast_to(wtc[:, :, None], (L, nch, N, 3, KCH, F))
    freqn = (np.arange(1, NRBF + 1, dtype=np.float32) * (PI / CUTOFF)
             / (2.0 * PI))
    shared = {
        "wbc": np.ascontiguousarray(wbc),
        "freqn": np.ascontiguousarray(np.tile(freqn[None, :], (N, 1))),
        "mw1": np.ascontiguousarray(p["msg_W1"].transpose(1, 0, 2)),
        "mb1": np.ascontiguousarray(p["msg_b1"].T),
        "mw2": np.ascontiguousarray(p["msg_W2"].transpose(1, 0, 2)),
        "mb2": np.ascontiguousarray(
            p["msg_b2"].reshape(L, 3, F).transpose(2, 0, 1)),
        "uwu": np.ascontiguousarray(p["upd_Wu"].transpose(1, 0, 2)),
        "uwv": np.ascontiguousarray(p["upd_Wv"].transpose(1, 0, 2)),
        "uws1": np.ascontiguousarray(
            p["upd_Ws1"].reshape(L, 2, F, F).transpose(2, 0, 1, 3)),
        "ubs1": np.ascontiguousarray(p["upd_bs1"].T),
        "uws2": np.ascontiguousarray(p["upd_Ws2"].transpose(1, 0, 2)),
        "ubs2": np.ascontiguousarray(
            p["upd_bs2"].reshape(L, 3, F).transpose(2, 0, 1)),
        "hmw1": np.ascontiguousarray(p["mu_W1"]),
        "hmb1": np.ascontiguousarray(p["mu_b1"][:, None]),
        "hmw2": np.ascontiguousarray(p["mu_W2"]),
        "hmb2": np.ascontiguousarray(p["mu_b2"][:, None]),
        "hsw1": np.ascontiguousarray(p["sig_W1"]),
        "hsb1": np.ascontiguousarray(p["sig_b1"][:, None]),
        "hsw2": np.ascontiguousarray(p["sig_W2"]),
        "hsb2": np.ascontiguousarray(p["sig_b2"][:, None]),
    }
    Ha = np.asarray(H, dtype=np.float32)
    Aa = np.asarray(cg_adj, dtype=np.float32)
    Xa = np.asarray(cg_xyz, dtype=np.float32)
    in_maps = []
    for c in range(N_CORES):
        b = c % B
        m = dict(shared)
        m["H"] = np.ascontiguousarray(Ha[b])
        m["adj"] = np.ascontiguousarray(Aa[b])
        m["xyz"] = np.ascontiguousarray(Xa[b])
        in_maps.append(m)
    return in_maps


_CACHED = {}


def _get_nc():
    if "nc" not in _CACHED:
        import concourse.bacc as bacc
        nc = bacc.Bacc("TRN2", target_bir_lowering=False, debug=False)
        build_program(nc)
        if not nc.is_finalized():
            nc.finalize()
        _CACHED["nc"] = nc
    return _CACHED["nc"]


def kernel(H, cg_adj, cg_xyz, params, _trace=False):
    from concourse.bass_utils import run_bass_kernel_spmd

    nc = _get_nc()
    in_maps = _host_prep(H, cg_adj, cg_xyz, params)
    res = run_bass_kernel_spmd(nc, in_maps, core_ids=list(range(N_CORES)),
                               trace=_trace)
    mu = np.stack([res.results[b]["out_mu"] for b in range(B)])
    sig = np.stack([res.results[b]["out_sig"] for b in range(B)])
    if _trace:
        kernel.last_exec_time_ns = res.exec_time_ns
    return (mu, sig)


if __name__ == "__main__":
    nc = _get_nc()
    print("built ok")


# revision 62
# speedup vs baseline: 1.1099x; 1.0539x over previous
"""Trainium2 Bass kernel for DenseCGPrior (PaiNN-style CG message passing).

Self-contained: hardcodes B=4, N=128, F=128, N_RBF=20, CUTOFF=5.0, L=3.
Sharding: data-parallel over batch; core c computes batch c % 4 (cores 4-7
duplicate so all 8 cores run the same SPMD program).

Key restructuring vs the reference: the [N,N,3F] per-edge message tensor is
never materialized. With 21 radial channels (20 RBF + 1 bias carrying rbf_b),
   inv[i,j,f'] = phi[j,f'] * sum_k g_k[i,j] * W[k,f']
so each edge reduction becomes 21 PSUM-accumulated matmuls with the symmetric
geometry matrices G_k (or G_k/d for the unit-vector term, which is decomposed
via unit = (x_j - x_i)/d into two matmul families plus rank-1 corrections).
"""

import os
import sys

import numpy as np

for _p in ("/opt/trn_rl_repo", "/root/.axon_site/_ro/trn_rl_repo"):
    if os.path.isdir(_p) and _p not in sys.path:
        sys.path.insert(0, _p)

import concourse.bass as bass
import concourse.mybir as mybir
import concourse.tile as tile
from concourse.masks import make_identity

F32 = mybir.dt.float32
F32R = mybir.dt.float32r
AF = mybir.ActivationFunctionType
OP = mybir.AluOpType

B, N, F, NRBF, L = 4, 128, 128, 20, 3
KC = NRBF + 1            # rbf channels + bias channel
KCH = 7                  # k-chunk size (21 = 3 chunks of 7)
F3 = 3 * F
EPS = 0.001
PI = float(np.pi)
CUTOFF = 5.0
N_CORES = 8

_IN_SPECS = [
    ("H", [N, F]),
    ("adj", [N, N]),
    ("xyz", [N, 3]),
    ("wbc", [L, 3, N, 3, KCH, F]),  # folded rbf weights, k-chunked,
                                    # replicated per partition
    ("freqn", [N, NRBF]),        # freq_k/(2pi) replicated per partition
    ("mw1", [F, L, F]),          # msg_W1 transposed to [f_in, l, f_out]
    ("mb1", [F, L]),
    ("mw2", [F, L, F3]),
    ("mb2", [F, L, 3]),          # [f, l, part]
    ("uwu", [F, L, F]),
    ("uwv", [F, L, F]),
    ("uws1", [F, L, 2, F]),      # [row_in_chunk, l, chunk, f_out]
    ("ubs1", [F, L]),
    ("uws2", [F, L, F3]),
    ("ubs2", [F, L, 3]),
    ("hmw1", [F, F]), ("hmb1", [F, 1]),
    ("hmw2", [F, F]), ("hmb2", [F, 1]),
    ("hsw1", [F, F]), ("hsb1", [F, 1]),
    ("hsw2", [F, F]), ("hsb2", [F, 1]),
]


def _rep(ap, times):
    """Read-broadcast a [P, M] AP as [P, times, M] via a step-0 free dim."""
    return bass.AP(tensor=ap.tensor, offset=ap.offset,
                   ap=[ap.ap[0], [0, times], *ap.ap[1:]])


_F32R_W = ("mw1", "mw2", "uwu", "uwv", "uws1", "uws2",
           "hmw1", "hmw2", "hsw1", "hsw2")


def build_program(nc):
    dins = {name: nc.dram_tensor(name, shape,
                                 F32R if name in _F32R_W else F32,
                                 kind="ExternalInput")
            for name, shape in _IN_SPECS}
    out_mu = nc.dram_tensor("out_mu", [N, F], F32, kind="ExternalOutput")
    out_sig = nc.dram_tensor("out_sig", [N, F], F32, kind="ExternalOutput")

    from contextlib import ExitStack
    with tile.TileContext(nc) as tc, ExitStack() as ctx:
        consts = ctx.enter_context(tc.tile_pool(name="consts", bufs=1))
        geom = ctx.enter_context(tc.tile_pool(name="geom", bufs=1))
        state = ctx.enter_context(tc.tile_pool(name="state", bufs=1))
        work = ctx.enter_context(tc.tile_pool(name="work", bufs=2))
        fams = ctx.enter_context(tc.tile_pool(name="fams", bufs=2))
        wbcp = ctx.enter_context(tc.tile_pool(name="wbcp", bufs=18))
        pp = ctx.enter_context(tc.tile_pool(name="pp", bufs=2, space="PSUM"))

        # ---- constants / weights to SBUF ----
        ident = consts.tile([N, N], F32)
        make_identity(nc, ident[:])
        # PE warmup on ident: absorbs the Pool-sem wait so later transposes
        # carry at most one sync wait (walrus LW struct limit).
        ps_wu = pp.tile([N, N], F32, tag="tr")
        nc.tensor.transpose(ps_wu[:], ident[:], ident[:])

        sb = {}
        _early = ("xyz", "adj", "freqn", "mw1", "mb1", "mw2", "mb2")

        def _load(names):
            for name, shape in _IN_SPECS:
                if name in ("H", "wbc") or name in sb:
                    continue
                if names is not None and name not in names:
                    continue
                t = consts.tile(shape, F32R if name in _F32R_W else F32,
                                tag=f"w_{name}")
                nc.sync.dma_start(t[:], dins[name].ap())
                sb[name] = t

        def load_wbc(l):
            """DMA layer-l folded rbf weights; one tile per (chunk, part),
            issued part-2 first (part 2 gates the o2 group, part 1 the o1
            slot-0 fold, part 0 only the l>0 dv1 folds)."""
            wbs = [[None] * 3 for _ in range(KC // KCH)]
            for p in (2, 1, 0):
                for ch in range(KC // KCH):
                    wb = wbcp.tile([N, KCH, F], F32, tag="wbc")
                    nc.sync.dma_start(wb[:],
                                      dins["wbc"].ap()[l, ch][:, p, :, :])
                    wbs[ch][p] = wb
            return wbs

        _load(_early)
        # H feeds sT which gates every matmul of layer 0: load it first.
        h0 = work.tile([N, F], F32, tag="h0")
        nc.sync.dma_start(h0[:], dins["H"].ap())
        # wbc of layer 0 gates the first message matmuls: issue right after
        # the small early weights.
        wbc0 = load_wbc(0)

        def ccol(val, tag):
            t = consts.tile([N, 1], F32, tag=tag)
            nc.vector.memset(t[:], val)
            return t

        c_eps = ccol(EPS, "c_eps")
        c_halfpi = ccol(PI / 2, "c_halfpi")

        def silu(out_t, in_ps, biascol, ztag):
            z = work.tile([F, N], F32, tag=ztag)
            nc.vector.tensor_scalar_add(z[:], in_ps, biascol)
            nc.scalar.activation(out_t, z[:], AF.Sigmoid)
            nc.vector.tensor_mul(out_t, z[:], out_t)

        # ---- geometry ----
        xyzt = sb["xyz"]
        adjt = sb["adj"]

        # xyzT [3, N] and -2*xyzT
        xyz_s = geom.tile([N, 3], F32)
        nc.vector.tensor_copy(xyz_s[:], xyzt[:])
        ps_x = pp.tile([3, N], F32, tag="tr")
        nc.tensor.transpose(ps_x[:], xyz_s[:], ident[:])
        xyzT = geom.tile([3, N], F32)
        nc.vector.tensor_copy(xyzT[:], ps_x[:])
        xyzTm2 = geom.tile([3, N], F32)
        nc.scalar.mul(xyzTm2[:], xyzT[:], -2.0)
        sqT = geom.tile([3, N], F32)
        nc.vector.tensor_mul(sqT[:], xyzT[:], xyzT[:])
        ones31 = geom.tile([3, 1], F32)
        nc.vector.memset(ones31[:], 1.0)
        ones1N = geom.tile([1, N], F32)
        nc.vector.memset(ones1N[:], 1.0)
        ps_nn = pp.tile([1, N], F32, tag="tr")
        nc.tensor.matmul(ps_nn[:], ones31[:], sqT[:], start=True, stop=True)
        nn_row = geom.tile([1, N], F32)
        nc.vector.tensor_copy(nn_row[:], ps_nn[:])

        # d2 = |xi|^2 + |xj|^2 - 2<xi,xj>  (PSUM accumulation)
        ps_d2 = pp.tile([N, N], F32, tag="tr")
        nc.tensor.matmul(ps_d2[:], xyzTm2[:], xyzT[:], start=True, stop=False)
        nc.tensor.matmul(ps_d2[:], ones1N[:], nn_row[:], start=False, stop=False)
        nc.tensor.matmul(ps_d2[:], nn_row[:], ones1N[:], start=False, stop=True)
        dmat = geom.tile([N, N], F32)
        nc.scalar.activation(dmat[:], ps_d2[:], AF.Sqrt, bias=c_eps[:])
        invd = geom.tile([N, N], F32)
        nc.vector.reciprocal(invd[:], dmat[:])

        # deg / dis / ew
        deg = geom.tile([N, 1], F32)
        nc.vector.reduce_sum(deg[:], adjt[:], axis=mybir.AxisListType.X)
        dis = geom.tile([N, 1], F32)
        nc.vector.reciprocal(dis[:], deg[:])
        nc.scalar.activation(dis[:], dis[:], AF.Sqrt, bias=c_eps[:])
        dis_s = geom.tile([N, 1], F32)
        nc.vector.tensor_copy(dis_s[:], dis[:])
        ps_dr = pp.tile([1, N], F32, tag="tr")
        nc.tensor.transpose(ps_dr[:], dis_s[:], ident[:])
        dis_row = geom.tile([1, N], F32)
        nc.vector.tensor_copy(dis_row[:], ps_dr[:])
        ps_ew = pp.tile([N, N], F32, tag="tr")
        nc.tensor.matmul(ps_ew[:], dis_row[:], dis_row[:], start=True, stop=True)
        mask = geom.tile([N, N], F32)
        nc.vector.tensor_scalar(mask[:], adjt[:], 0.0, None, op0=OP.is_gt)
        ew = geom.tile([N, N], F32)
        nc.vector.tensor_mul(ew[:], mask[:], ps_ew[:])

        # envelope: env = 0.5 + 0.5*sin(pi/2 - pi*min(d,CUTOFF)/CUTOFF)
        dc = geom.tile([N, N], F32)
        nc.vector.tensor_scalar_min(dc[:], dmat[:], CUTOFF)
        env = geom.tile([N, N], F32)
        nc.scalar.activation(env[:], dc[:], AF.Sin,
                             bias=c_halfpi[:], scale=-PI / CUTOFF)
        nc.vector.tensor_scalar(env[:], env[:], 0.5, 0.5,
                                op0=OP.mult, op1=OP.add)
        eew = geom.tile([N, N], F32)
        nc.vector.tensor_mul(eew[:], env[:], ew[:])
        sh1 = geom.tile([N, N], F32)    # env*ew/d
        nc.vector.tensor_mul(sh1[:], eew[:], invd[:])

        # G[j, k, i]: k<20 -> sin(d*freq_k)*env*ew/d ; k=20 -> env*ew
        # sin range reduction: t = d*freq_k/(2pi); frac = t - rne(t) in
        # [-1/2, 1/2]; sin(d*freq_k) = sin(2pi*frac). rne via the exact
        # float trick (t + 1.5*2^23) - 1.5*2^23, identical on DVE and numpy.
        G = geom.tile([N, KC, N], F32R)
        GD = geom.tile([N, KC, N], F32R)
        RC = 12582912.0  # 1.5 * 2^23
        fq = sb["freqn"]  # [N, NRBF] per-partition copies of freq_k/(2pi)
        fq_ap = bass.AP(tensor=fq[:].tensor, offset=fq[:].offset,
                        ap=[fq[:].ap[0], fq[:].ap[1], [0, N]])
        # Built in k-chunks of 7 so the first message matmuls can start as
        # soon as chunk 0 is ready. f32 scratches borrowed from the fam pool
        # (distinct memrefs, so the fp32r producer-rounding check is happy).
        for ch in range(KC // KCH):
            k0 = ch * KCH
            nk = min(KCH, NRBF - k0)  # last chunk: 6 rbf rows + bias row
            scr1 = fams.tile([N, KCH, N], F32, tag="fam1")
            scr2 = fams.tile([N, KCH, N], F32, tag="fam2")
            fqa = bass.AP(tensor=fq[:].tensor, offset=fq[:].offset + k0,
                          ap=[fq[:].ap[0], [1, nk], [0, N]])
            nc.vector.tensor_tensor(scr1[:, 0:nk, :], _rep(dmat[:], nk),
                                    fqa, op=OP.mult)
            nc.vector.tensor_scalar(scr2[:, 0:nk, :], scr1[:, 0:nk, :],
                                    RC, RC, op0=OP.add, op1=OP.subtract)
            nc.vector.tensor_tensor(scr1[:, 0:nk, :], scr1[:, 0:nk, :],
                                    scr2[:, 0:nk, :], op=OP.subtract)
            nc.scalar.activation(scr1[:, 0:nk, :], scr1[:, 0:nk, :],
                                 AF.Sin, scale=2.0 * PI)
            nc.vector.tensor_tensor(G[:, k0:k0 + nk, :], scr1[:, 0:nk, :],
                                    _rep(sh1[:], nk), op=OP.mult)
            if ch == KC // KCH - 1:
                nc.vector.tensor_copy(G[:, NRBF, :], eew[:])
            nc.vector.tensor_tensor(GD[:, k0:k0 + KCH, :],
                                    G[:, k0:k0 + KCH, :].bitcast(F32),
                                    _rep(invd[:], KCH), op=OP.mult)

        # ---- initial state ----
        sT = state.tile([F, N], F32R)       # s transposed [f, n]
        ps_s0 = pp.tile([N, N], F32, tag="tr")
        h0s = work.tile([N, F], F32, tag="h0s")
        nc.vector.tensor_copy(h0s[:], h0[:])
        nc.tensor.transpose(ps_s0[:], h0s[:], ident[:])
        nc.vector.tensor_copy(sT[:], ps_s0[:])
        v = state.tile([N, 3, F], F32)     # v[n, c, f]

        _load(None)  # remaining weights, lower DMA priority

        for l in range(L):
            wbc = wbc0 if l == 0 else load_wbc(l)
            # ---- phi = silu(s@W1 + b1) @ W2 + b2, produced transposed ----
            ps1 = pp.tile([F, N], F32, tag="mm")
            nc.tensor.matmul(ps1[:], sb["mw1"][:, l, :], sT[:],
                             start=True, stop=True)
            act1 = work.tile([F, N], F32R, tag="act1")
            silu(act1[:], ps1[:], sb["mb1"][:, l:l + 1], "z1")
            phi = [None, None, None]
            for p in (2, 1, 0):   # part 2 first: it gates the o2 group
                ps2 = pp.tile([F, N], F32, tag="mm")
                nc.tensor.matmul(ps2[:], sb["mw2"][:, l, p * F:(p + 1) * F],
                                 act1[:], start=True, stop=True)
                phiT_p = work.tile([F, N], F32, tag=f"phiT{p}")
                nc.vector.tensor_scalar_add(phiT_p[:], ps2[:],
                                            sb["mb2"][:, l, p:p + 1])
                ps_t = pp.tile([N, F], F32, tag="tr")
                nc.tensor.transpose(ps_t[:], phiT_p[:], ident[:])
                phi_p = work.tile([N, F], F32, tag=f"phi{p}")
                nc.vector.tensor_copy(phi_p[:], ps_t[:])
                phi[p] = phi_p

            pv = []
            if l > 0:
                for c in range(3):
                    pv_c = work.tile([N, F], F32, tag=f"pv{c}")
                    nc.vector.tensor_mul(pv_c[:], phi[0][:], v[:, c, :])
                    pv.append(pv_c)

            # ---- message matmuls: 21 channels in 3 chunks of 7 ----
            nslot1 = 1 if l == 0 else 4
            o1 = pp.tile([N, nslot1 * F], F32, tag="acc")
            o2 = pp.tile([N, 4 * F], F32, tag="acc")
            for ch in range(KC // KCH):
                k0 = ch * KCH
                fam2 = fams.tile([N, KCH, 4, F], F32R, tag="fam2")
                nc.vector.tensor_tensor(fam2[:, :, 0, :],
                                        _rep(phi[2][:], KCH),
                                        wbc[ch][2][:], op=OP.mult)
                for c in range(3):
                    nc.scalar.mul(fam2[:, :, 1 + c, :], fam2[:, :, 0, :],
                                  xyzt[:, c:c + 1])
                fam1 = fams.tile([N, KCH, nslot1, F], F32R, tag="fam1")
                nc.gpsimd.tensor_tensor(fam1[:, :, 0, :],
                                        _rep(phi[1][:], KCH),
                                        wbc[ch][1][:], op=OP.mult)
                for c in range(3) if l > 0 else ():
                    nc.vector.tensor_tensor(fam1[:, :, 1 + c, :],
                                            _rep(pv[c][:], KCH),
                                            wbc[ch][0][:],
                                            op=OP.mult)
                for kk in range(KCH):
                    k = k0 + kk
                    nc.tensor.matmul(
                        o2[:], GD[:, k, :],
                        fam2[:, kk, :, :].rearrange("p s f -> p (s f)"),
                        start=(k == 0), stop=(k == KC - 1))
                for kk in range(KCH):
                    k = k0 + kk
                    nc.tensor.matmul(
                        o1[:], G[:, k, :],
                        fam1[:, kk, :, :].rearrange("p s f -> p (s f)"),
                        start=(k == 0), stop=(k == KC - 1))

            # ---- apply ds / dv ----
            ds = work.tile([N, F], F32, tag="ds")
            nc.vector.tensor_copy(ds[:], o1[:, 0:F])
            ps_dst = pp.tile([F, N], F32, tag="tr")
            nc.tensor.transpose(ps_dst[:], ds[:], ident[:])
            nc.vector.tensor_add(sT[:], sT[:], ps_dst[:])

            for c in range(3):
                qx = work.tile([N, F], F32, tag=f"qx{c}")
                nc.vector.tensor_scalar_mul(qx[:], o2[:, 0:F],
                                            xyzt[:, c:c + 1])
                lo = (1 + c) * F
                if l == 0:
                    nc.vector.tensor_sub(v[:, c, :], o2[:, lo:lo + F], qx[:])
                else:
                    nc.vector.tensor_sub(qx[:], o2[:, lo:lo + F], qx[:])
                    nc.vector.tensor_add(qx[:], qx[:], o1[:, lo:lo + F])
                    nc.vector.tensor_add(v[:, c, :], v[:, c, :], qx[:])

            # ---- update block (PaiNN) ----
            vT = []
            for c in range(3):
                ps_vt = pp.tile([F, N], F32, tag="tr")
                nc.tensor.transpose(ps_vt[:], v[:, c, :], ident[:])
                vT_c = work.tile([F, N], F32R, tag=f"vT{c}")
                nc.vector.tensor_copy(vT_c[:], ps_vt[:])
                vT.append(vT_c)
            ps_uv = pp.tile([F, 3, N], F32, tag="uv")
            ps_vv = pp.tile([F, 3, N], F32, tag="uv")
            for c in range(3):
                nc.tensor.matmul(ps_uv[:, c, :], sb["uwu"][:, l, :], vT[c][:],
                                 start=True, stop=True)
            for c in range(3):
                nc.tensor.matmul(ps_vv[:, c, :], sb["uwv"][:, l, :], vT[c][:],
                                 start=True, stop=True)

            uvs = work.tile([F, 3, N], F32, tag="uvs", bufs=1)
            nc.vector.tensor_copy(uvs[:], ps_uv[:])
            vvs = work.tile([F, 3, N], F32, tag="vvs", bufs=1)
            nc.vector.tensor_copy(vvs[:], ps_vv[:])
            vvn = work.tile([F, N], F32R, tag="vvn")
            nc.vector.tensor_mul(vvn[:], vvs[:, 0, :], vvs[:, 0, :])
            dot = work.tile([F, N], F32, tag="dot")
            nc.vector.tensor_mul(dot[:], uvs[:, 0, :], vvs[:, 0, :])
            tq = work.tile([F, N], F32, tag="tq")
            for c in (1, 2):
                nc.vector.tensor_mul(tq[:], vvs[:, c, :], vvs[:, c, :])
                nc.vector.tensor_add(vvn[:], vvn[:], tq[:])
                nc.vector.tensor_mul(tq[:], uvs[:, c, :], vvs[:, c, :])
                nc.vector.tensor_add(dot[:], dot[:], tq[:])
            nc.scalar.activation(vvn[:], vvn[:], AF.Sqrt, bias=c_eps[:])
            # dep-pinned PE filler: keeps the HAM activity window alive
            # through this serial DVE/ACT stretch (output never read)
            ps_f1 = pp.tile([N, F], F32, tag="tr")
            nc.tensor.transpose(ps_f1[:], dot[:], ident[:])

            ps3 = pp.tile([F, N], F32, tag="mm")
            nc.tensor.matmul(ps3[:], sb["uws1"][:, l, 0, :], sT[:],
                             start=True, stop=False)
            nc.tensor.matmul(ps3[:], sb["uws1"][:, l, 1, :], vvn[:],
                             start=False, stop=True)
            act2 = work.tile([F, N], F32R, tag="act2")
            silu(act2[:], ps3[:], sb["ubs1"][:, l:l + 1], "z2")
            ps_f2 = pp.tile([N, F], F32, tag="tr")
            nc.tensor.transpose(ps_f2[:], act2[:].bitcast(F32), ident[:])
            # s path first: it gates the next layer's phi matmuls
            aT = [None, None, None]
            for p in (1, 2, 0):
                ps4 = pp.tile([F, N], F32, tag="mm")
                nc.tensor.matmul(ps4[:], sb["uws2"][:, l, p * F:(p + 1) * F],
                                 act2[:], start=True, stop=True)
                aT_p = work.tile([F, N], F32, tag=f"aT{p}")
                nc.scalar.activation(aT_p[:], ps4[:], AF.Identity,
                                     bias=sb["ubs2"][:, l, p:p + 1])
                aT[p] = aT_p
                if p == 2:
                    # ds_u = a_sv * dot + a_ss ; sT += ds_uT
                    nc.vector.tensor_mul(dot[:], dot[:], aT[1][:])
                    nc.vector.tensor_add(dot[:], dot[:], aT[2][:])
                    nc.vector.tensor_add(sT[:], sT[:], dot[:])

            # dv_u = u_v * a_vv ; v += dv_u (via transpose back)
            for c in range(3):
                dvuT = work.tile([F, N], F32, tag=f"dvuT{c}")
                nc.vector.tensor_mul(dvuT[:], uvs[:, c, :], aT[0][:])
                ps_b = pp.tile([N, F], F32, tag="tr")
                nc.tensor.transpose(ps_b[:], dvuT[:], ident[:])
                nc.vector.tensor_add(v[:, c, :], v[:, c, :], ps_b[:])

        # ---- output heads ----
        def head(w1, b1, w2, b2, out_dram, is_sigma):
            psh = pp.tile([F, N], F32, tag="mm")
            nc.tensor.matmul(psh[:], sb[w1][:], sT[:], start=True, stop=True)
            th = work.tile([F, N], F32R, tag="head_t")
            nc.scalar.activation(th[:], psh[:], AF.Tanh, bias=sb[b1][:])
            psh2 = pp.tile([F, N], F32, tag="mm")
            nc.tensor.matmul(psh2[:], sb[w2][:], th[:], start=True, stop=True)
            hT = work.tile([F, N], F32, tag="head_o")
            # bias-add on DVE so the transpose input's last writer is DVE
            nc.vector.tensor_scalar_add(hT[:], psh2[:], sb[b2][:])
            ps_o = pp.tile([N, F], F32, tag="tr")
            nc.tensor.transpose(ps_o[:], hT[:], ident[:])
            ho = work.tile([N, F], F32, tag="head_f")
            if is_sigma:
                # sigma = 1e-9 + exp((x + b2)/2)
                nc.scalar.activation(ho[:], ps_o[:], AF.Exp, scale=0.5)
                nc.vector.tensor_scalar_add(ho[:], ho[:], 1e-9)
            else:
                nc.vector.tensor_copy(ho[:], ps_o[:])
            nc.sync.dma_start(out_dram.ap(), ho[:])

        head("hmw1", "hmb1", "hmw2", "hmb2", out_mu, False)
        head("hsw1", "hsb1", "hsw2", "hsb2", out_sig, True)

    return nc


def _host_prep(H, cg_adj, cg_xyz, params):
    p = {k: np.asarray(v, dtype=np.float32) for k, v in params.items()}
    wtil = np.concatenate([p["rbf_W"], p["rbf_b"][:, None, :]], axis=1)
    # [L, KC, 3F] -> [L, CH, 3, KCH, F] -> replicate partitions
    nch = KC // KCH
    wtc = wtil.reshape(L, nch, KCH, 3, F).transpose(0, 1, 3, 2, 4)
    wbc = np.broadcast_to(wtc[:, :, None], (L, nch, N, 3, KCH, F))
    freqn = (np.arange(1, NRBF + 1, dtype=np.float32) * (PI / CUTOFF)
             / (2.0 * PI))
    shared = {
        "wbc": np.ascontiguousarray(wbc),
        "freqn": np.ascontiguousarray(np.tile(freqn[None, :], (N, 1))),
        "mw1": np.ascontiguousarray(p["msg_W1"].transpose(1, 0, 2)),
        "mb1": np.ascontiguousarray(p["msg_b1"].T),
        "mw2": np.ascontiguousarray(p["msg_W2"].transpose(1, 0, 2)),
        "mb2": np.ascontiguousarray(
            p["msg_b2"].reshape(L, 3, F).transpose(2, 0, 1)),
        "uwu": np.ascontiguousarray(p["upd_Wu"].transpose(1, 0, 2)),
        "uwv": np.ascontiguousarray(p["upd_Wv"].transpose(1, 0, 2)),
        "uws1": np.ascontiguousarray(
            p["upd_Ws1"].reshape(L, 2, F, F).transpose(2, 0, 1, 3)),
        "ubs1": np.ascontiguousarray(p["upd_bs1"].T),
        "uws2": np.ascontiguousarray(p["upd_Ws2"].transpose(1, 0, 2)),
        "ubs2": np.ascontiguousarray(
            p["upd_bs2"].reshape(L, 3, F).transpose(2, 0, 1)),
        "hmw1": np.ascontiguousarray(p["mu_W1"]),
        "hmb1": np.ascontiguousarray(p["mu_b1"][:, None]),
        "hmw2": np.ascontiguousarray(p["mu_W2"]),
        "hmb2": np.ascontiguousarray(p["mu_b2"][:, None]),
        "hsw1": np.ascontiguousarray(p["sig_W1"]),
        "hsb1": np.ascontiguousarray(p["sig_b1"][:, None]),
        "hsw2": np.ascontiguousarray(p["sig_W2"]),
        "hsb2": np.ascontiguousarray(p["sig_b2"][:, None]),
    }
    Ha = np.asarray(H, dtype=np.float32)
    Aa = np.asarray(cg_adj, dtype=np.float32)
    Xa = np.asarray(cg_xyz, dtype=np.float32)
    in_maps = []
    for c in range(N_CORES):
        b = c % B
        m = dict(shared)
        m["H"] = np.ascontiguousarray(Ha[b])
        m["adj"] = np.ascontiguousarray(Aa[b])
        m["xyz"] = np.ascontiguousarray(Xa[b])
        in_maps.append(m)
    return in_maps


_CACHED = {}


def _get_nc():
    if "nc" not in _CACHED:
        import concourse.bacc as bacc
        nc = bacc.Bacc("TRN2", target_bir_lowering=False, debug=False)
        build_program(nc)
        if not nc.is_finalized():
            nc.finalize()
        _CACHED["nc"] = nc
    return _CACHED["nc"]


def kernel(H, cg_adj, cg_xyz, params, _trace=False):
    from concourse.bass_utils import run_bass_kernel_spmd

    nc = _get_nc()
    in_maps = _host_prep(H, cg_adj, cg_xyz, params)
    res = run_bass_kernel_spmd(nc, in_maps, core_ids=list(range(N_CORES)),
                               trace=_trace)
    mu = np.stack([res.results[b]["out_mu"] for b in range(B)])
    sig = np.stack([res.results[b]["out_sig"] for b in range(B)])
    if _trace:
        kernel.last_exec_time_ns = res.exec_time_ns
    return (mu, sig)


if __name__ == "__main__":
    nc = _get_nc()
    print("built ok")
